# revision 7
# baseline (speedup 1.0000x reference)
"""Adder2D (L1-distance conv) Trainium2 kernel, data-parallel over batch on 8 cores.

out[n,h,w,f] = bias[f] - sum_{i,j,c} |x_pad[n, h+i, w+j, c] - kernel[i,j,c,f]|

Per-core shapes (batch 32 sharded 8 ways): x [4,32,32,128], kernel [3,3,128,128],
bias [128], out [4,32,32,128].

Decomposition used on-chip (avoids any abs op):
    |t| = t + 2*relu(-t)  with  t = x - w
 => sum_k |x-w| = sum_k x - sum_k w - 2*sum_k min(x-w, 0)
The channel dim C=128 sits on SBUF partitions; the 3x3 offsets are shifted views
of a zero-padded channels-first image. Per (filter, offset) one DVE
tensor_scalar(subtract, min) pass produces min(x-w,0) tiles in bf16 (4x mode);
the PE reduces over partitions with a (-2)-valued one-hot stationary,
accumulating into PSUM. sum_k x uses an all-ones stationary (filter-independent)
and sum_k w + bias folds into the per-filter bias applied when draining PSUM.
"""

import sys

if "/opt/trn_rl_repo" not in sys.path:
    sys.path.insert(0, "/opt/trn_rl_repo")

from contextlib import ExitStack

import numpy as np

import concourse.bass as bass  # noqa: F401
import concourse.tile as tile
from concourse import bacc, mybir
from concourse.bass_utils import run_bass_kernel_spmd
from concourse.masks import make_identity

AL = mybir.AluOpType
DT = mybir.dt
AF = mybir.ActivationFunctionType

N_CORES = 8
NL = 4  # images per core
H = W = 32
C = 128
F = 128
PH, PW = 34, 36  # padded rows / padded row pitch (even pitch for alignment)
M = NL * H * W  # 4096 output positions per core
CH = 512  # matmul moving chunk (one PSUM bank of fp32)
NCH = M // CH  # 8
NPAD = NL * PH * PW  # flat padded row length per partition (4896)

OFFS = [(i, j) for i in range(3) for j in range(3)]

# How many of the 9 offsets are computed on ScalarE (activation Abs with
# per-partition bias) instead of VectorE. ACT-handled offsets contribute
# sum|x-w| directly via a +1 one-hot stationary and skip the sum(x) stream.
ACT_OFFS = 2
# Column-tiling width for the PE reduction: 4 concurrent 128x32 matmuls.
NSTRIP = 4
FG = F // NSTRIP  # filter groups (32)


def _body(tc, o_d, x_d, w_d, b_d):
    nc = tc.nc
    with ExitStack() as ctx:
        const = ctx.enter_context(tc.tile_pool(name="const", bufs=1))

        ident = const.tile([128, 128], DT.bfloat16)
        make_identity(nc, ident[:])
        ident32 = const.tile([128, 128], DT.float32)
        make_identity(nc, ident32[:])

        # stationary tiles: window zneg[:, 127-f:255-f] is a one-hot column f
        zneg = const.tile([128, 255], DT.bfloat16)  # -2 at col 127
        nc.vector.memset(zneg[:], 0.0)
        nc.vector.memset(zneg[:, 127:128], -2.0)
        zpos = const.tile([128, 255], DT.bfloat16)  # +1 at col 127 (ACT offsets)
        nc.vector.memset(zpos[:], 0.0)
        nc.vector.memset(zpos[:, 127:128], 1.0)
        ones_s = const.tile([128, 128], DT.bfloat16)  # all-ones stationary
        nc.vector.memset(ones_s[:], 1.0)
        ones_col = const.tile([128, 1], DT.float32)
        nc.vector.memset(ones_col[:], 1.0)

        # padded channels-first input
        xa = const.tile([128, NL, PH, PW], DT.bfloat16)
        xa_flat = xa[:].rearrange("p n r c -> p (n r c)")
        nc.vector.memset(xa_flat, 0.0)

        # weights [c, off, f] fp32 and combined bias B[f] = bias[f] + sum_k w[k,f]
        wt = const.tile([128, 9, 128], DT.float32)
        nc.sync.dma_start(wt[:], w_d.rearrange("i j c f -> c (i j) f"))
        bias_row = const.tile([1, 128], DT.float32)
        nc.sync.dma_start(bias_row[:], b_d[:])

        # sum_k w only over VectorE-handled offsets (ScalarE offsets feed
        # |x-w| directly and need no correction term)
        dve_w = [oi for oi in range(9) if oi not in set(range(9 - ACT_OFFS, 9))]
        wsum = const.tile([128, 128], DT.float32)
        nc.vector.tensor_tensor(wsum[:], wt[:, dve_w[0], :], wt[:, dve_w[1], :], AL.add)
        for o in dve_w[2:]:
            nc.vector.tensor_tensor(wsum[:], wsum[:], wt[:, o, :], AL.add)

        bsum = const.tile([128, 1], DT.float32)
        with tc.tile_pool(name="bp", bufs=1, space="PSUM") as bpp:
            bp = bpp.tile([128, 1], DT.float32)
            nc.tensor.matmul(bp[:], wsum[:], ones_col[:], start=True, stop=False)
            nc.tensor.matmul(
                bp[:], bias_row[:], ones_col[0:1, 0:1], start=False, stop=True
            )
            nc.vector.tensor_copy(bsum[:], bp[:])

        # ---- stage 1: DMA input, convert bf16, PE-transpose into padded buf
        with tc.tile_pool(name="s1", bufs=4) as s1, tc.tile_pool(
            name="s1p", bufs=4, space="PSUM"
        ) as s1p:
            x_flat = x_d.rearrange("n h w c -> (n h w) c")
            for t in range(M // 128):
                n, h0 = divmod(t, 8)
                h0 *= 4
                tf = s1.tile([128, 128], DT.float32)
                nc.sync.dma_start(tf[:], x_flat[t * 128 : (t + 1) * 128, :])
                tb = s1.tile([128, 128], DT.bfloat16)
                nc.vector.tensor_copy(tb[:], tf[:])
                pp = s1p.tile([128, 128], DT.bfloat16)
                nc.tensor.transpose(pp[:], tb[:], ident[:])
                nc.vector.tensor_copy(
                    xa[:, n, 1 + h0 : 1 + h0 + 4, 1:33],
                    pp[:].rearrange("p (a b) -> p a b", a=4),
                )
        # ---- main loop
        # offsets handled by ScalarE (true |x-w|, +1 one-hot) vs VectorE
        # (min(x-w,0), -2 one-hot + sum(x) correction streams)
        act_set = set(range(9 - ACT_OFFS, 9))
        dve_offs = [oi for oi in range(9) if oi not in act_set]
        act_offs = [oi for oi in range(9) if oi in act_set]
        with tc.tile_pool(name="mp", bufs=1, space="PSUM") as mp, tc.tile_pool(
            name="u", bufs=8
        ) as up:
            P = mp.tile([128, M], DT.float32)

            # sum_c x streams (ones stationary), only for DVE-handled offsets
            for k, oi in enumerate(dve_offs):
                i, j = OFFS[oi]
                for s in range(NCH):
                    n, h0 = divmod(s, 2)
                    nc.tensor.matmul(
                        P[:, s * CH : (s + 1) * CH],
                        ones_s[:],
                        xa[:, n, i + h0 * 16 : i + h0 * 16 + 16, j : j + 32],
                        start=(k == 0),
                        stop=False,
                    )

            # filter group g covers filters {g, g+32, g+64, g+96}; strip js
            # computes filter g + 32*js as a 128x32 column-tiled matmul whose
            # PSUM partition range starts at 32*js, so PSUM row == filter.
            for g in range(FG):
                for klass, offs in (("dve", dve_offs), ("act", act_offs)):
                    for oi in offs:
                        i, j = OFFS[oi]
                        xin = xa[:, :, i : i + 32, :].rearrange(
                            "p n r c -> p n (r c)"
                        )
                        us = []
                        for js in range(NSTRIP):
                            f = g + FG * js
                            wap = wt[:, oi, f : f + 1]
                            u = up.tile([128, NL, 32 * PW], DT.bfloat16)
                            if klass == "act":
                                nc.scalar.activation(
                                    u[:], xin, AF.Abs, bias=wap, scale=-1.0
                                )
                            else:
                                nc.vector.tensor_scalar(
                                    u[:], xin, wap, 0.0, AL.subtract, AL.min
                                )
                            us.append(u)
                        zwin = zpos if klass == "act" else zneg
                        stat = zwin[:, 127 - g : 159 - g]
                        last = g == FG - 1 and oi == (act_offs or dve_offs)[-1]
                        for s in range(NCH):
                            n, h0 = divmod(s, 2)
                            for js in range(NSTRIP):
                                u4 = us[js][:].rearrange(
                                    "p n (r c) -> p n r c", r=32
                                )
                                nc.tensor.matmul(
                                    P[32 * js : 32 * js + 32, s * CH : (s + 1) * CH],
                                    stat,
                                    u4[:, n, h0 * 16 : h0 * 16 + 16, j : j + 32],
                                    start=False,
                                    stop=last,
                                    tile_position=(0, 32 * js),
                                )

            # ---- drain: out = -P + (bias + sum w)
            sout = const.tile([128, M], DT.float32)
            for s in range(NCH):
                nc.scalar.activation(
                    sout[:, s * CH : (s + 1) * CH],
                    P[:, s * CH : (s + 1) * CH],
                    AF.Identity,
                    bias=bsum[:],
                    scale=-1.0,
                )

        # ---- transpose [f, m] -> [m, f] and store
        o_flat = o_d.rearrange("n h w f -> (n h w) f")
        with tc.tile_pool(name="tr", bufs=4) as trp, tc.tile_pool(
            name="trp", bufs=4, space="PSUM"
        ) as trpp:
            for t in range(M // 128):
                pt = trpp.tile([128, 128], DT.float32)
                nc.tensor.transpose(pt[:], sout[:, t * 128 : (t + 1) * 128], ident32[:])
                ot = trp.tile([128, 128], DT.float32)
                nc.vector.tensor_copy(ot[:], pt[:])
                nc.sync.dma_start(o_flat[t * 128 : (t + 1) * 128, :], ot[:])


_nc_cache = None


def _build():
    global _nc_cache
    if _nc_cache is None:
        nc = bacc.Bacc("TRN2", target_bir_lowering=False, debug=False, num_devices=N_CORES)
        x_d = nc.dram_tensor("inputs", [NL, H, W, C], DT.float32, kind="ExternalInput").ap()
        w_d = nc.dram_tensor("kernel", [3, 3, C, F], DT.float32, kind="ExternalInput").ap()
        b_d = nc.dram_tensor("bias", [1, F], DT.float32, kind="ExternalInput").ap()
        o_d = nc.dram_tensor("out", [NL, H, W, F], DT.float32, kind="ExternalOutput").ap()
        with tile.TileContext(nc) as tc:
            _body(tc, o_d, x_d, w_d, b_d)
        nc.compile()
        _nc_cache = nc
    return _nc_cache


def run(inputs, kernel, bias, **spmd_kwargs):
    nc = _build()
    shards = np.split(np.ascontiguousarray(inputs, dtype=np.float32), N_CORES, axis=0)
    kf = np.ascontiguousarray(kernel, dtype=np.float32)
    bf = np.ascontiguousarray(bias, dtype=np.float32).reshape(1, F)
    in_maps = [{"inputs": s, "kernel": kf, "bias": bf} for s in shards]
    res = run_bass_kernel_spmd(nc, in_maps, core_ids=list(range(N_CORES)), **spmd_kwargs)
    out = np.concatenate([r["out"] for r in res.results], axis=0)
    return out, res


def kernel(inputs, kernel, bias):
    out, _ = run(inputs, kernel, bias)
    return out


# revision 9
# speedup vs baseline: 1.3316x; 1.3316x over previous
"""Adder2D (L1-distance conv) Trainium2 kernel, data-parallel over batch on 8 cores.

out[n,h,w,f] = bias[f] - sum_{i,j,c} |x_pad[n, h+i, w+j, c] - kernel[i,j,c,f]|

Per-core shapes (batch 32 sharded 8 ways): x [4,32,32,128], kernel [3,3,128,128],
bias [128], out [4,32,32,128].

Decomposition used on-chip (avoids any abs op):
    |t| = t + 2*relu(-t)  with  t = x - w
 => sum_k |x-w| = sum_k x - sum_k w - 2*sum_k min(x-w, 0)
The channel dim C=128 sits on SBUF partitions; the 3x3 offsets are shifted views
of a zero-padded channels-first image. Per (filter, offset) one DVE
tensor_scalar(subtract, min) pass produces min(x-w,0) tiles in bf16 (4x mode);
the PE reduces over partitions with a (-2)-valued one-hot stationary,
accumulating into PSUM. sum_k x uses an all-ones stationary (filter-independent)
and sum_k w + bias folds into the per-filter bias applied when draining PSUM.
"""

import sys

if "/opt/trn_rl_repo" not in sys.path:
    sys.path.insert(0, "/opt/trn_rl_repo")

from contextlib import ExitStack

import numpy as np

import concourse.bass as bass  # noqa: F401
import concourse.tile as tile
from concourse import bacc, mybir
from concourse.bass_utils import run_bass_kernel_spmd
from concourse.masks import make_identity

AL = mybir.AluOpType
DT = mybir.dt
AF = mybir.ActivationFunctionType

N_CORES = 8
NL = 4  # images per core
H = W = 32
C = 128
F = 128
PH, PW = 34, 36  # padded rows / padded row pitch (even pitch for alignment)
M = NL * H * W  # 4096 output positions per core
CH = 512  # matmul moving chunk (one PSUM bank of fp32)
NCH = M // CH  # 8
NPAD = NL * PH * PW  # flat padded row length per partition (4896)

OFFS = [(i, j) for i in range(3) for j in range(3)]

# How many of the 9 offsets are computed on ScalarE (activation Abs with
# per-partition bias) instead of VectorE. ACT-handled offsets contribute
# sum|x-w| directly via a +1 one-hot stationary and skip the sum(x) stream.
ACT_OFFS = 2
# Column-tiling width for the PE reduction: 4 concurrent 128x32 matmuls.
NSTRIP = 4
FG = F // NSTRIP  # filter groups (32)


def _body(tc, o_d, x_d, w_d, b_d):
    nc = tc.nc
    with ExitStack() as ctx:
        const = ctx.enter_context(tc.tile_pool(name="const", bufs=1))

        ident = const.tile([128, 128], DT.bfloat16)
        make_identity(nc, ident[:])
        ident32 = const.tile([128, 128], DT.float32)
        make_identity(nc, ident32[:])

        # stationary tiles: window zneg[:, 127-f:255-f] is a one-hot column f
        zneg = const.tile([128, 255], DT.bfloat16)  # -2 at col 127
        nc.vector.memset(zneg[:], 0.0)
        nc.vector.memset(zneg[:, 127:128], -2.0)
        zpos = const.tile([128, 255], DT.bfloat16)  # +1 at col 127 (ACT offsets)
        nc.vector.memset(zpos[:], 0.0)
        nc.vector.memset(zpos[:, 127:128], 1.0)
        ones_s = const.tile([128, 128], DT.bfloat16)  # all-ones stationary
        nc.vector.memset(ones_s[:], 1.0)
        ones_col = const.tile([128, 1], DT.float32)
        nc.vector.memset(ones_col[:], 1.0)

        # padded channels-first input
        xa = const.tile([128, NL, PH, PW], DT.bfloat16)
        xa_flat = xa[:].rearrange("p n r c -> p (n r c)")
        nc.vector.memset(xa_flat, 0.0)

        # weights [c, off, f] fp32 and combined bias B[f] = bias[f] + sum_k w[k,f]
        wt = const.tile([128, 9, 128], DT.float32)
        nc.sync.dma_start(wt[:], w_d.rearrange("i j c f -> c (i j) f"))
        bias_row = const.tile([1, 128], DT.float32)
        nc.sync.dma_start(bias_row[:], b_d[:])

        # sum_k w only over VectorE-handled offsets (ScalarE offsets feed
        # |x-w| directly and need no correction term)
        dve_w = [oi for oi in range(9) if oi not in set(range(9 - ACT_OFFS, 9))]
        wsum = const.tile([128, 128], DT.float32)
        nc.vector.tensor_tensor(wsum[:], wt[:, dve_w[0], :], wt[:, dve_w[1], :], AL.add)
        for o in dve_w[2:]:
            nc.vector.tensor_tensor(wsum[:], wsum[:], wt[:, o, :], AL.add)

        bsum = const.tile([128, 1], DT.float32)
        with tc.tile_pool(name="bp", bufs=1, space="PSUM") as bpp:
            bp = bpp.tile([128, 1], DT.float32)
            nc.tensor.matmul(bp[:], wsum[:], ones_col[:], start=True, stop=False)
            nc.tensor.matmul(
                bp[:], bias_row[:], ones_col[0:1, 0:1], start=False, stop=True
            )
            nc.vector.tensor_copy(bsum[:], bp[:])

        # ---- stage 1: DMA input, convert bf16, PE-transpose into padded buf
        with tc.tile_pool(name="s1", bufs=4) as s1, tc.tile_pool(
            name="s1p", bufs=4, space="PSUM"
        ) as s1p:
            x_flat = x_d.rearrange("n h w c -> (n h w) c")
            for t in range(M // 128):
                n, h0 = divmod(t, 8)
                h0 *= 4
                tf = s1.tile([128, 128], DT.float32)
                nc.sync.dma_start(tf[:], x_flat[t * 128 : (t + 1) * 128, :])
                tb = s1.tile([128, 128], DT.bfloat16)
                nc.vector.tensor_copy(tb[:], tf[:])
                pp = s1p.tile([128, 128], DT.bfloat16)
                nc.tensor.transpose(pp[:], tb[:], ident[:])
                nc.vector.tensor_copy(
                    xa[:, n, 1 + h0 : 1 + h0 + 4, 1:33],
                    pp[:].rearrange("p (a b) -> p a b", a=4),
                )
        # ---- main loop
        # offsets handled by ScalarE (true |x-w|, +1 one-hot) vs VectorE
        # (min(x-w,0), -2 one-hot + sum(x) correction streams)
        act_set = set(range(9 - ACT_OFFS, 9))
        dve_offs = [oi for oi in range(9) if oi not in act_set]
        act_offs = [oi for oi in range(9) if oi in act_set]
        with tc.tile_pool(name="mp", bufs=1, space="PSUM") as mp, tc.tile_pool(
            name="udve", bufs=10
        ) as updve, tc.tile_pool(name="uact", bufs=5) as upact:
            P = mp.tile([128, M], DT.float32)

            # sum_c x streams (ones stationary), only for DVE-handled offsets
            for k, oi in enumerate(dve_offs):
                i, j = OFFS[oi]
                for s in range(NCH):
                    n, h0 = divmod(s, 2)
                    nc.tensor.matmul(
                        P[:, s * CH : (s + 1) * CH],
                        ones_s[:],
                        xa[:, n, i + h0 * 16 : i + h0 * 16 + 16, j : j + 32],
                        start=(k == 0),
                        stop=False,
                    )

            # filter group g covers filters {g, g+32, g+64, g+96}; strip js
            # computes filter g + 32*js as a 128x32 column-tiled matmul whose
            # PSUM partition range starts at 32*js, so PSUM row == filter.
            for g in range(FG):
                for klass, offs in (("dve", dve_offs), ("act", act_offs)):
                    for oi in offs:
                        i, j = OFFS[oi]
                        xin = xa[:, :, i : i + 32, :].rearrange(
                            "p n r c -> p n (r c)"
                        )
                        us = []
                        for js in range(NSTRIP):
                            f = g + FG * js
                            wap = wt[:, oi, f : f + 1]
                            pool = upact if klass == "act" else updve
                            u = pool.tile([128, NL, 32 * PW], DT.bfloat16)
                            if klass == "act":
                                nc.scalar.activation(
                                    u[:], xin, AF.Abs, bias=wap, scale=-1.0
                                )
                            else:
                                nc.vector.tensor_scalar(
                                    u[:], xin, wap, 0.0, AL.subtract, AL.min
                                )
                            us.append(u)
                        zwin = zpos if klass == "act" else zneg
                        stat = zwin[:, 127 - g : 159 - g]
                        last = g == FG - 1 and oi == (act_offs or dve_offs)[-1]
                        for s in range(NCH):
                            n, h0 = divmod(s, 2)
                            for js in range(NSTRIP):
                                u4 = us[js][:].rearrange(
                                    "p n (r c) -> p n r c", r=32
                                )
                                nc.tensor.matmul(
                                    P[32 * js : 32 * js + 32, s * CH : (s + 1) * CH],
                                    stat,
                                    u4[:, n, h0 * 16 : h0 * 16 + 16, j : j + 32],
                                    start=False,
                                    stop=last,
                                    tile_position=(0, 32 * js),
                                )

            # ---- drain: out = -P + (bias + sum w)
            sout = const.tile([128, M], DT.float32)
            for s in range(NCH):
                nc.scalar.activation(
                    sout[:, s * CH : (s + 1) * CH],
                    P[:, s * CH : (s + 1) * CH],
                    AF.Identity,
                    bias=bsum[:],
                    scale=-1.0,
                )

        # ---- transpose [f, m] -> [m, f] and store
        o_flat = o_d.rearrange("n h w f -> (n h w) f")
        with tc.tile_pool(name="tr", bufs=4) as trp, tc.tile_pool(
            name="trp", bufs=4, space="PSUM"
        ) as trpp:
            for t in range(M // 128):
                pt = trpp.tile([128, 128], DT.float32)
                nc.tensor.transpose(pt[:], sout[:, t * 128 : (t + 1) * 128], ident32[:])
                ot = trp.tile([128, 128], DT.float32)
                nc.vector.tensor_copy(ot[:], pt[:])
                nc.sync.dma_start(o_flat[t * 128 : (t + 1) * 128, :], ot[:])


_nc_cache = None


def _build():
    global _nc_cache
    if _nc_cache is None:
        nc = bacc.Bacc("TRN2", target_bir_lowering=False, debug=False, num_devices=N_CORES)
        x_d = nc.dram_tensor("inputs", [NL, H, W, C], DT.float32, kind="ExternalInput").ap()
        w_d = nc.dram_tensor("kernel", [3, 3, C, F], DT.float32, kind="ExternalInput").ap()
        b_d = nc.dram_tensor("bias", [1, F], DT.float32, kind="ExternalInput").ap()
        o_d = nc.dram_tensor("out", [NL, H, W, F], DT.float32, kind="ExternalOutput").ap()
        with tile.TileContext(nc) as tc:
            _body(tc, o_d, x_d, w_d, b_d)
        nc.compile()
        _nc_cache = nc
    return _nc_cache


def run(inputs, kernel, bias, **spmd_kwargs):
    nc = _build()
    shards = np.split(np.ascontiguousarray(inputs, dtype=np.float32), N_CORES, axis=0)
    kf = np.ascontiguousarray(kernel, dtype=np.float32)
    bf = np.ascontiguousarray(bias, dtype=np.float32).reshape(1, F)
    in_maps = [{"inputs": s, "kernel": kf, "bias": bf} for s in shards]
    res = run_bass_kernel_spmd(nc, in_maps, core_ids=list(range(N_CORES)), **spmd_kwargs)
    out = np.concatenate([r["out"] for r in res.results], axis=0)
    return out, res


def kernel(inputs, kernel, bias):
    out, _ = run(inputs, kernel, bias)
    return out


# revision 12
# speedup vs baseline: 1.5794x; 1.1861x over previous
"""Adder2D (L1-distance conv) Trainium2 kernel, data-parallel over batch on 8 cores.

out[n,h,w,f] = bias[f] - sum_{i,j,c} |x_pad[n, h+i, w+j, c] - kernel[i,j,c,f]|

Per-core shapes (batch 32 sharded 8 ways): x [4,32,32,128], kernel [3,3,128,128],
bias [128], out [4,32,32,128].

Decomposition used on-chip (avoids any abs op):
    |t| = t + 2*relu(-t)  with  t = x - w
 => sum_k |x-w| = sum_k x - sum_k w - 2*sum_k min(x-w, 0)
The channel dim C=128 sits on SBUF partitions; the 3x3 offsets are shifted views
of a zero-padded channels-first image. Per (filter, offset) one DVE
tensor_scalar(subtract, min) pass produces min(x-w,0) tiles in bf16 (4x mode);
the PE reduces over partitions with a (-2)-valued one-hot stationary,
accumulating into PSUM. sum_k x uses an all-ones stationary (filter-independent)
and sum_k w + bias folds into the per-filter bias applied when draining PSUM.
"""

import sys

if "/opt/trn_rl_repo" not in sys.path:
    sys.path.insert(0, "/opt/trn_rl_repo")

from contextlib import ExitStack

import numpy as np

import concourse.bass as bass  # noqa: F401
import concourse.tile as tile
from concourse import bacc, mybir
from concourse.bass_utils import run_bass_kernel_spmd
from concourse.masks import make_identity

AL = mybir.AluOpType
DT = mybir.dt
AF = mybir.ActivationFunctionType

N_CORES = 8
NL = 4  # images per core
H = W = 32
C = 128
F = 128
PH, PW = 34, 36  # padded rows / padded row pitch (even pitch for alignment)
M = NL * H * W  # 4096 output positions per core
CH = 512  # matmul moving chunk (one PSUM bank of fp32)
NCH = M // CH  # 8
NPAD = NL * PH * PW  # flat padded row length per partition (4896)

OFFS = [(i, j) for i in range(3) for j in range(3)]

# How many of the 9 offsets are computed on ScalarE (activation Abs with
# per-partition bias) instead of VectorE. ACT-handled offsets contribute
# sum|x-w| directly via a +1 one-hot stationary and skip the sum(x) stream.
ACT_OFFS = 2
# Column-tiling width for the PE reduction: 4 concurrent 128x32 matmuls.
NSTRIP = 4
FG = F // NSTRIP  # filter groups (32)


def _body(tc, o_d, x_d, w_d, b_d):
    nc = tc.nc
    with ExitStack() as ctx:
        const = ctx.enter_context(tc.tile_pool(name="const", bufs=1))

        ident = const.tile([128, 128], DT.bfloat16)
        make_identity(nc, ident[:])
        ident32 = const.tile([128, 128], DT.float32)
        make_identity(nc, ident32[:])

        # stationary tiles: window zneg[:, 127-f:255-f] is a one-hot column f
        zneg = const.tile([128, 255], DT.bfloat16)  # -2 at col 127
        nc.vector.memset(zneg[:], 0.0)
        nc.vector.memset(zneg[:, 127:128], -2.0)
        zpos = const.tile([128, 255], DT.bfloat16)  # +1 at col 127 (ACT offsets)
        nc.vector.memset(zpos[:], 0.0)
        nc.vector.memset(zpos[:, 127:128], 1.0)
        ones_s = const.tile([128, 128], DT.bfloat16)  # all-ones stationary
        nc.vector.memset(ones_s[:], 1.0)
        ones_col = const.tile([128, 1], DT.float32)
        nc.vector.memset(ones_col[:], 1.0)

        # padded channels-first input
        xa = const.tile([128, NL, PH, PW], DT.bfloat16)
        xa_flat = xa[:].rearrange("p n r c -> p (n r c)")
        nc.vector.memset(xa_flat, 0.0)

        # weights [c, off, f] fp32 and combined bias B[f] = bias[f] + sum_k w[k,f]
        wt = const.tile([128, 9, 128], DT.float32)
        nc.sync.dma_start(wt[:], w_d.rearrange("i j c f -> c (i j) f"))
        bias_row = const.tile([1, 128], DT.float32)
        nc.sync.dma_start(bias_row[:], b_d[:])

        # sum_k w only over VectorE-handled offsets (ScalarE offsets feed
        # |x-w| directly and need no correction term)
        dve_w = [oi for oi in range(9) if oi not in set(range(9 - ACT_OFFS, 9))]
        wsum = const.tile([128, 128], DT.float32)
        nc.vector.tensor_tensor(wsum[:], wt[:, dve_w[0], :], wt[:, dve_w[1], :], AL.add)
        for o in dve_w[2:]:
            nc.vector.tensor_tensor(wsum[:], wsum[:], wt[:, o, :], AL.add)

        bsum = const.tile([128, 1], DT.float32)
        with tc.tile_pool(name="bp", bufs=1, space="PSUM") as bpp:
            bp = bpp.tile([128, 1], DT.float32)
            nc.tensor.matmul(bp[:], wsum[:], ones_col[:], start=True, stop=False)
            nc.tensor.matmul(
                bp[:], bias_row[:], ones_col[0:1, 0:1], start=False, stop=True
            )
            nc.vector.tensor_copy(bsum[:], bp[:])

        # ---- stage 1: DMA input, convert bf16, PE-transpose into padded buf
        with tc.tile_pool(name="s1", bufs=4) as s1, tc.tile_pool(
            name="s1p", bufs=4, space="PSUM"
        ) as s1p:
            x_flat = x_d.rearrange("n h w c -> (n h w) c")
            for t in range(M // 128):
                n, h0 = divmod(t, 8)
                h0 *= 4
                tf = s1.tile([128, 128], DT.float32)
                nc.sync.dma_start(tf[:], x_flat[t * 128 : (t + 1) * 128, :])
                tb = s1.tile([128, 128], DT.bfloat16)
                nc.vector.tensor_copy(tb[:], tf[:])
                pp = s1p.tile([128, 128], DT.bfloat16)
                nc.tensor.transpose(pp[:], tb[:], ident[:])
                nc.vector.tensor_copy(
                    xa[:, n, 1 + h0 : 1 + h0 + 4, 1:33],
                    pp[:].rearrange("p (a b) -> p a b", a=4),
                )
        # ---- main loop
        # offsets handled by ScalarE (true |x-w|, +1 one-hot) vs VectorE
        # (min(x-w,0), -2 one-hot + sum(x) correction streams)
        act_set = set(range(9 - ACT_OFFS, 9))
        dve_offs = [oi for oi in range(9) if oi not in act_set]
        act_offs = [oi for oi in range(9) if oi in act_set]
        with tc.tile_pool(name="mp", bufs=1, space="PSUM") as mp, tc.tile_pool(
            name="udve", bufs=9
        ) as updve, tc.tile_pool(name="uact", bufs=6) as upact:
            P = mp.tile([128, M], DT.float32)

            # sum_c x streams (ones stationary), only for DVE-handled offsets
            for k, oi in enumerate(dve_offs):
                i, j = OFFS[oi]
                for s in range(NCH):
                    n, h0 = divmod(s, 2)
                    nc.tensor.matmul(
                        P[:, s * CH : (s + 1) * CH],
                        ones_s[:],
                        xa[:, n, i + h0 * 16 : i + h0 * 16 + 16, j : j + 32],
                        start=(k == 0),
                        stop=False,
                    )

            # filter group g covers filters {g, g+32, g+64, g+96}; strip js
            # computes filter g + 32*js as a 128x32 column-tiled matmul whose
            # PSUM partition range starts at 32*js, so PSUM row == filter.
            # Interleave ScalarE-class offsets among VectorE-class ones so the
            # PE queue roughly matches production-completion order.
            plan = list(dve_offs)
            for k, oi in enumerate(act_offs):
                plan.insert(3 + k * 4, oi)
            for g in range(FG):
                for oi in plan:
                    klass = "act" if oi in act_set else "dve"
                    if True:
                        i, j = OFFS[oi]
                        xin = xa[:, :, i : i + 32, :].rearrange(
                            "p n r c -> p n (r c)"
                        )
                        us = []
                        for js in range(NSTRIP):
                            f = g + FG * js
                            wap = wt[:, oi, f : f + 1]
                            pool = upact if klass == "act" else updve
                            u = pool.tile([128, NL, 32 * PW], DT.bfloat16)
                            if klass == "act":
                                nc.scalar.activation(
                                    u[:], xin, AF.Abs, bias=wap, scale=-1.0
                                )
                            else:
                                nc.vector.tensor_scalar(
                                    u[:], xin, wap, 0.0, AL.subtract, AL.min
                                )
                            us.append(u)
                        zwin = zpos if klass == "act" else zneg
                        stat = zwin[:, 127 - g : 159 - g]
                        last = g == FG - 1 and oi == plan[-1]
                        for s in range(NCH):
                            n, h0 = divmod(s, 2)
                            for js in range(NSTRIP):
                                u4 = us[js][:].rearrange(
                                    "p n (r c) -> p n r c", r=32
                                )
                                nc.tensor.matmul(
                                    P[32 * js : 32 * js + 32, s * CH : (s + 1) * CH],
                                    stat,
                                    u4[:, n, h0 * 16 : h0 * 16 + 16, j : j + 32],
                                    start=False,
                                    stop=last,
                                    tile_position=(0, 32 * js),
                                )

            # ---- drain: out = -P + (bias + sum w)
            sout = const.tile([128, M], DT.float32)
            for s in range(NCH):
                nc.scalar.activation(
                    sout[:, s * CH : (s + 1) * CH],
                    P[:, s * CH : (s + 1) * CH],
                    AF.Identity,
                    bias=bsum[:],
                    scale=-1.0,
                )

        # ---- transpose [f, m] -> [m, f] and store
        o_flat = o_d.rearrange("n h w f -> (n h w) f")
        with tc.tile_pool(name="tr", bufs=4) as trp, tc.tile_pool(
            name="trp", bufs=4, space="PSUM"
        ) as trpp:
            for t in range(M // 128):
                pt = trpp.tile([128, 128], DT.float32)
                nc.tensor.transpose(pt[:], sout[:, t * 128 : (t + 1) * 128], ident32[:])
                ot = trp.tile([128, 128], DT.float32)
                nc.vector.tensor_copy(ot[:], pt[:])
                nc.sync.dma_start(o_flat[t * 128 : (t + 1) * 128, :], ot[:])


_nc_cache = None


def _build():
    global _nc_cache
    if _nc_cache is None:
        nc = bacc.Bacc("TRN2", target_bir_lowering=False, debug=False, num_devices=N_CORES)
        x_d = nc.dram_tensor("inputs", [NL, H, W, C], DT.float32, kind="ExternalInput").ap()
        w_d = nc.dram_tensor("kernel", [3, 3, C, F], DT.float32, kind="ExternalInput").ap()
        b_d = nc.dram_tensor("bias", [1, F], DT.float32, kind="ExternalInput").ap()
        o_d = nc.dram_tensor("out", [NL, H, W, F], DT.float32, kind="ExternalOutput").ap()
        with tile.TileContext(nc) as tc:
            _body(tc, o_d, x_d, w_d, b_d)
        nc.compile()
        _nc_cache = nc
    return _nc_cache


def run(inputs, kernel, bias, **spmd_kwargs):
    nc = _build()
    shards = np.split(np.ascontiguousarray(inputs, dtype=np.float32), N_CORES, axis=0)
    kf = np.ascontiguousarray(kernel, dtype=np.float32)
    bf = np.ascontiguousarray(bias, dtype=np.float32).reshape(1, F)
    in_maps = [{"inputs": s, "kernel": kf, "bias": bf} for s in shards]
    res = run_bass_kernel_spmd(nc, in_maps, core_ids=list(range(N_CORES)), **spmd_kwargs)
    out = np.concatenate([r["out"] for r in res.results], axis=0)
    return out, res


def kernel(inputs, kernel, bias):
    out, _ = run(inputs, kernel, bias)
    return out


# revision 17
# speedup vs baseline: 1.7100x; 1.0827x over previous
"""Adder2D (L1-distance conv) Trainium2 kernel, data-parallel over batch on 8 cores.

out[n,h,w,f] = bias[f] - sum_{i,j,c} |x_pad[n, h+i, w+j, c] - kernel[i,j,c,f]|

Per-core shapes (batch 32 sharded 8 ways): x [4,32,32,128], kernel [3,3,128,128],
bias [128], out [4,32,32,128].

Decomposition used on-chip (avoids any abs op):
    |t| = t + 2*relu(-t)  with  t = x - w
 => sum_k |x-w| = sum_k x - sum_k w - 2*sum_k min(x-w, 0)
The channel dim C=128 sits on SBUF partitions; the 3x3 offsets are shifted views
of a zero-padded channels-first image. Per (filter, offset) one DVE
tensor_scalar(subtract, min) pass produces min(x-w,0) tiles in bf16 (4x mode);
the PE reduces over partitions with a (-2)-valued one-hot stationary,
accumulating into PSUM. sum_k x uses an all-ones stationary (filter-independent)
and sum_k w + bias folds into the per-filter bias applied when draining PSUM.
"""

import sys

if "/opt/trn_rl_repo" not in sys.path:
    sys.path.insert(0, "/opt/trn_rl_repo")

from contextlib import ExitStack

import numpy as np

import concourse.bass as bass  # noqa: F401
import concourse.tile as tile
from concourse import bacc, mybir
from concourse.bass_utils import run_bass_kernel_spmd
from concourse.masks import make_identity

AL = mybir.AluOpType
DT = mybir.dt
AF = mybir.ActivationFunctionType

N_CORES = 8
NL = 4  # images per core
H = W = 32
C = 128
F = 128
PH, PW = 34, 34  # padded rows / padded row pitch
M = NL * H * W  # 4096 output positions per core
CH = 512  # matmul moving chunk (one PSUM bank of fp32)
NCH = M // CH  # 8
NPAD = NL * PH * PW  # flat padded row length per partition (4896)

OFFS = [(i, j) for i in range(3) for j in range(3)]

# How many of the 9 offsets are computed on ScalarE (activation Abs with
# per-partition bias) instead of VectorE. ACT-handled offsets contribute
# sum|x-w| directly via a +1 one-hot stationary and skip the sum(x) stream.
ACT_OFFS = 2
# Column-tiling width for the PE reduction: 4 concurrent 128x32 matmuls.
NSTRIP = 4
FG = F // NSTRIP  # filter groups (32)


def _body(tc, o_d, x_d, w_d, b_d):
    nc = tc.nc
    with ExitStack() as ctx:
        const = ctx.enter_context(tc.tile_pool(name="const", bufs=1))

        ident = const.tile([128, 128], DT.bfloat16)
        make_identity(nc, ident[:])
        ident32 = const.tile([128, 128], DT.float32)
        make_identity(nc, ident32[:])

        # stationary tiles: window zneg[:, 127-f:255-f] is a one-hot column f
        zneg = const.tile([128, 255], DT.bfloat16)  # -2 at col 127
        nc.vector.memset(zneg[:], 0.0)
        nc.vector.memset(zneg[:, 127:128], -2.0)
        zpos = const.tile([128, 255], DT.bfloat16)  # +1 at col 127 (ACT offsets)
        nc.vector.memset(zpos[:], 0.0)
        nc.vector.memset(zpos[:, 127:128], 1.0)
        ones_s = const.tile([128, 128], DT.bfloat16)  # all-ones stationary
        nc.vector.memset(ones_s[:], 1.0)
        # ones over columns 0..96 only: sum(x) stream stationary for the
        # mixed offset whose strip 3 is handled by ScalarE
        ones96 = const.tile([128, 128], DT.bfloat16)
        nc.vector.memset(ones96[:], 0.0)
        nc.vector.memset(ones96[:, 0:96], 1.0)
        ones_col = const.tile([128, 1], DT.float32)
        nc.vector.memset(ones_col[:], 1.0)

        # padded channels-first input
        xa = const.tile([128, NL, PH, PW], DT.bfloat16)
        xa_flat = xa[:].rearrange("p n r c -> p (n r c)")
        nc.vector.memset(xa_flat, 0.0)

        # weights [c, off, f] fp32 and combined bias B[f] = bias[f] + sum_k w[k,f]
        wt = const.tile([128, 9, 128], DT.float32)
        nc.sync.dma_start(wt[:], w_d.rearrange("i j c f -> c (i j) f"))
        bias_row = const.tile([1, 128], DT.float32)
        nc.sync.dma_start(bias_row[:], b_d[:])

        # sum_k w only over VectorE-handled (offset, filter) units (ScalarE
        # units feed |x-w| directly and need no correction term). The mixed
        # offset MIX_OFF is VectorE for filters 0..95 and ScalarE for 96..127.
        dve_w = [oi for oi in range(9) if oi not in set(range(9 - ACT_OFFS, 9))]
        mix_off = dve_w[-1]
        wsum = const.tile([128, 128], DT.float32)
        nc.vector.tensor_tensor(wsum[:], wt[:, dve_w[0], :], wt[:, dve_w[1], :], AL.add)
        for o in dve_w[2:-1]:
            nc.vector.tensor_tensor(wsum[:], wsum[:], wt[:, o, :], AL.add)
        nc.vector.tensor_tensor(
            wsum[:, 0:96], wsum[:, 0:96], wt[:, mix_off, 0:96], AL.add
        )

        bsum = const.tile([128, 1], DT.float32)
        with tc.tile_pool(name="bp", bufs=1, space="PSUM") as bpp:
            bp = bpp.tile([128, 1], DT.float32)
            nc.tensor.matmul(bp[:], wsum[:], ones_col[:], start=True, stop=False)
            nc.tensor.matmul(
                bp[:], bias_row[:], ones_col[0:1, 0:1], start=False, stop=True
            )
            nc.vector.tensor_copy(bsum[:], bp[:])

        # ---- stage 1: DMA input, convert bf16, PE-transpose into padded buf
        with tc.tile_pool(name="s1", bufs=4) as s1, tc.tile_pool(
            name="s1p", bufs=4, space="PSUM"
        ) as s1p:
            x_flat = x_d.rearrange("n h w c -> (n h w) c")
            for t in range(M // 128):
                n, h0 = divmod(t, 8)
                h0 *= 4
                tf = s1.tile([128, 128], DT.float32)
                nc.sync.dma_start(tf[:], x_flat[t * 128 : (t + 1) * 128, :])
                tb = s1.tile([128, 128], DT.bfloat16)
                nc.vector.tensor_copy(tb[:], tf[:])
                pp = s1p.tile([128, 128], DT.bfloat16)
                nc.tensor.transpose(pp[:], tb[:], ident[:])
                nc.vector.tensor_copy(
                    xa[:, n, 1 + h0 : 1 + h0 + 4, 1:33],
                    pp[:].rearrange("p (a b) -> p a b", a=4),
                )
        # ---- main loop
        # offsets handled by ScalarE (true |x-w|, +1 one-hot) vs VectorE
        # (min(x-w,0), -2 one-hot + sum(x) correction streams)
        act_set = set(range(9 - ACT_OFFS, 9))
        dve_offs = [oi for oi in range(9) if oi not in act_set]
        act_offs = [oi for oi in range(9) if oi in act_set]
        with tc.tile_pool(name="mp", bufs=1, space="PSUM") as mp, tc.tile_pool(
            name="udve", bufs=9
        ) as updve, tc.tile_pool(name="uact", bufs=6) as upact:
            P = mp.tile([128, M], DT.float32)

            # sum_c x streams (ones stationary), only for DVE-handled offsets;
            # the mixed offset covers only filter rows 0..95
            for k, oi in enumerate(dve_offs):
                i, j = OFFS[oi]
                for s in range(NCH):
                    n, h0 = divmod(s, 2)
                    nc.tensor.matmul(
                        P[:, s * CH : (s + 1) * CH],
                        ones96[:] if oi == mix_off else ones_s[:],
                        xa[:, n, i + h0 * 16 : i + h0 * 16 + 16, j : j + 32],
                        start=(k == 0),
                        stop=False,
                    )

            # filter group g covers filters {g, g+32, g+64, g+96}; strip js
            # computes filter g + 32*js as a 128x32 column-tiled matmul whose
            # PSUM partition range starts at 32*js, so PSUM row == filter.
            # Interleave ScalarE-class offsets among VectorE-class ones so the
            # PE queue roughly matches production-completion order.
            plan = list(dve_offs)
            for k, oi in enumerate(act_offs):
                plan.insert(3 + k * 4, oi)
            for g in range(FG):
                for oi in plan:
                    klass = "act" if oi in act_set else "dve"
                    if True:
                        i, j = OFFS[oi]
                        xin = xa[:, :, i : i + 32, :].rearrange(
                            "p n r c -> p n (r c)"
                        )
                        us = []
                        kls = []
                        for js in range(NSTRIP):
                            f = g + FG * js
                            wap = wt[:, oi, f : f + 1]
                            k2 = klass
                            if oi == mix_off and js == NSTRIP - 1:
                                k2 = "act"  # rebalance: ScalarE takes strip 3
                            pool = upact if k2 == "act" else updve
                            u = pool.tile([128, NL, 32 * PW], DT.bfloat16)
                            if k2 == "act":
                                nc.scalar.activation(
                                    u[:], xin, AF.Abs, bias=wap, scale=-1.0
                                )
                            else:
                                nc.vector.tensor_scalar(
                                    u[:], xin, wap, 0.0, AL.subtract, AL.min
                                )
                            us.append(u)
                            kls.append(k2)
                        last = g == FG - 1 and oi == plan[-1]
                        for s in range(NCH):
                            n, h0 = divmod(s, 2)
                            for js in range(NSTRIP):
                                zwin = zpos if kls[js] == "act" else zneg
                                stat = zwin[:, 127 - g : 159 - g]
                                u4 = us[js][:].rearrange(
                                    "p n (r c) -> p n r c", r=32
                                )
                                nc.tensor.matmul(
                                    P[32 * js : 32 * js + 32, s * CH : (s + 1) * CH],
                                    stat,
                                    u4[:, n, h0 * 16 : h0 * 16 + 16, j : j + 32],
                                    start=False,
                                    stop=last,
                                    tile_position=(0, 32 * js),
                                )

            # ---- drain: out = -P + (bias + sum w)
            sout = const.tile([128, M], DT.float32)
            for s in range(NCH):
                nc.scalar.activation(
                    sout[:, s * CH : (s + 1) * CH],
                    P[:, s * CH : (s + 1) * CH],
                    AF.Identity,
                    bias=bsum[:],
                    scale=-1.0,
                )

        # ---- transpose [f, m] -> [m, f] and store
        o_flat = o_d.rearrange("n h w f -> (n h w) f")
        with tc.tile_pool(name="tr", bufs=4) as trp, tc.tile_pool(
            name="trp", bufs=4, space="PSUM"
        ) as trpp:
            for t in range(M // 128):
                pt = trpp.tile([128, 128], DT.float32)
                nc.tensor.transpose(pt[:], sout[:, t * 128 : (t + 1) * 128], ident32[:])
                ot = trp.tile([128, 128], DT.float32)
                nc.vector.tensor_copy(ot[:], pt[:])
                nc.sync.dma_start(o_flat[t * 128 : (t + 1) * 128, :], ot[:])


_nc_cache = None


def _build():
    global _nc_cache
    if _nc_cache is None:
        nc = bacc.Bacc("TRN2", target_bir_lowering=False, debug=False, num_devices=N_CORES)
        x_d = nc.dram_tensor("inputs", [NL, H, W, C], DT.float32, kind="ExternalInput").ap()
        w_d = nc.dram_tensor("kernel", [3, 3, C, F], DT.float32, kind="ExternalInput").ap()
        b_d = nc.dram_tensor("bias", [1, F], DT.float32, kind="ExternalInput").ap()
        o_d = nc.dram_tensor("out", [NL, H, W, F], DT.float32, kind="ExternalOutput").ap()
        with tile.TileContext(nc) as tc:
            _body(tc, o_d, x_d, w_d, b_d)
        nc.compile()
        _nc_cache = nc
    return _nc_cache


def run(inputs, kernel, bias, **spmd_kwargs):
    nc = _build()
    shards = np.split(np.ascontiguousarray(inputs, dtype=np.float32), N_CORES, axis=0)
    kf = np.ascontiguousarray(kernel, dtype=np.float32)
    bf = np.ascontiguousarray(bias, dtype=np.float32).reshape(1, F)
    in_maps = [{"inputs": s, "kernel": kf, "bias": bf} for s in shards]
    res = run_bass_kernel_spmd(nc, in_maps, core_ids=list(range(N_CORES)), **spmd_kwargs)
    out = np.concatenate([r["out"] for r in res.results], axis=0)
    return out, res


def kernel(inputs, kernel, bias):
    out, _ = run(inputs, kernel, bias)
    return out


# revision 20
# speedup vs baseline: 1.7116x; 1.0009x over previous
"""Adder2D (L1-distance conv) Trainium2 kernel, data-parallel over batch on 8 cores.

out[n,h,w,f] = bias[f] - sum_{i,j,c} |x_pad[n, h+i, w+j, c] - kernel[i,j,c,f]|

Per-core shapes (batch 32 sharded 8 ways): x [4,32,32,128], kernel [3,3,128,128],
bias [128], out [4,32,32,128].

Decomposition used on-chip (avoids any abs op):
    |t| = t + 2*relu(-t)  with  t = x - w
 => sum_k |x-w| = sum_k x - sum_k w - 2*sum_k min(x-w, 0)
The channel dim C=128 sits on SBUF partitions; the 3x3 offsets are shifted views
of a zero-padded channels-first image. Per (filter, offset) one DVE
tensor_scalar(subtract, min) pass produces min(x-w,0) tiles in bf16 (4x mode);
the PE reduces over partitions with a (-2)-valued one-hot stationary,
accumulating into PSUM. sum_k x uses an all-ones stationary (filter-independent)
and sum_k w + bias folds into the per-filter bias applied when draining PSUM.
"""

import sys

if "/opt/trn_rl_repo" not in sys.path:
    sys.path.insert(0, "/opt/trn_rl_repo")

from contextlib import ExitStack

import numpy as np

import concourse.bass as bass  # noqa: F401
import concourse.tile as tile
from concourse import bacc, mybir
from concourse.bass_utils import run_bass_kernel_spmd
from concourse.masks import make_identity

AL = mybir.AluOpType
DT = mybir.dt
AF = mybir.ActivationFunctionType

N_CORES = 8
NL = 4  # images per core
H = W = 32
C = 128
F = 128
PH, PW = 34, 34  # padded rows / padded row pitch
M = NL * H * W  # 4096 output positions per core
CH = 512  # matmul moving chunk (one PSUM bank of fp32)
NCH = M // CH  # 8
NPAD = NL * PH * PW  # flat padded row length per partition (4896)

OFFS = [(i, j) for i in range(3) for j in range(3)]

# How many of the 9 offsets are computed on ScalarE (activation Abs with
# per-partition bias) instead of VectorE. ACT-handled offsets contribute
# sum|x-w| directly via a +1 one-hot stationary and skip the sum(x) stream.
ACT_OFFS = 2
# Column-tiling width for the PE reduction: 4 concurrent 128x32 matmuls.
NSTRIP = 4
FG = F // NSTRIP  # filter groups (32)


def _body(tc, o_d, x_d, w_d, b_d):
    nc = tc.nc
    with ExitStack() as ctx:
        const = ctx.enter_context(tc.tile_pool(name="const", bufs=1))

        ident = const.tile([128, 128], DT.bfloat16)
        make_identity(nc, ident[:])
        ident32 = const.tile([128, 128], DT.float32)
        make_identity(nc, ident32[:])

        # stationary tiles: window zneg[:, 127-f:255-f] is a one-hot column f
        zneg = const.tile([128, 255], DT.bfloat16)  # -2 at col 127
        nc.vector.memset(zneg[:], 0.0)
        nc.vector.memset(zneg[:, 127:128], -2.0)
        zpos = const.tile([128, 255], DT.bfloat16)  # +1 at col 127 (ACT offsets)
        nc.vector.memset(zpos[:], 0.0)
        nc.vector.memset(zpos[:, 127:128], 1.0)
        ones_s = const.tile([128, 128], DT.bfloat16)  # all-ones stationary
        nc.vector.memset(ones_s[:], 1.0)
        # ones over columns 0..96 only: sum(x) stream stationary for the
        # mixed offset whose strip 3 is handled by ScalarE
        ones96 = const.tile([128, 128], DT.bfloat16)
        nc.vector.memset(ones96[:], 0.0)
        nc.vector.memset(ones96[:, 0:96], 1.0)
        ones_col = const.tile([128, 1], DT.float32)
        nc.vector.memset(ones_col[:], 1.0)

        # padded channels-first input
        xa = const.tile([128, NL, PH, PW], DT.bfloat16)
        xa_flat = xa[:].rearrange("p n r c -> p (n r c)")
        nc.vector.memset(xa_flat, 0.0)

        # weights [c, off, f] fp32 and combined bias B[f] = bias[f] + sum_k w[k,f]
        wt = const.tile([128, 9, 128], DT.float32)
        nc.sync.dma_start(wt[:], w_d.rearrange("i j c f -> c (i j) f"))
        bias_row = const.tile([1, 128], DT.float32)
        nc.sync.dma_start(bias_row[:], b_d[:])

        # sum_k w only over VectorE-handled (offset, filter) units (ScalarE
        # units feed |x-w| directly and need no correction term). The mixed
        # offset MIX_OFF is VectorE for filters 0..95 and ScalarE for 96..127.
        dve_w = [oi for oi in range(9) if oi not in set(range(9 - ACT_OFFS, 9))]
        mix_off = dve_w[-1]
        wsum = const.tile([128, 128], DT.float32)
        nc.vector.tensor_tensor(wsum[:], wt[:, dve_w[0], :], wt[:, dve_w[1], :], AL.add)
        for o in dve_w[2:-1]:
            nc.vector.tensor_tensor(wsum[:], wsum[:], wt[:, o, :], AL.add)
        nc.vector.tensor_tensor(
            wsum[:, 0:96], wsum[:, 0:96], wt[:, mix_off, 0:96], AL.add
        )

        bsum = const.tile([128, 1], DT.float32)
        with tc.tile_pool(name="bp", bufs=1, space="PSUM") as bpp:
            bp = bpp.tile([128, 1], DT.float32)
            nc.tensor.matmul(bp[:], wsum[:], ones_col[:], start=True, stop=False)
            nc.tensor.matmul(
                bp[:], bias_row[:], ones_col[0:1, 0:1], start=False, stop=True
            )
            nc.vector.tensor_copy(bsum[:], bp[:])

        # ---- stage 1: DMA input, convert bf16, PE-transpose into padded buf
        with tc.tile_pool(name="s1", bufs=4) as s1, tc.tile_pool(
            name="s1p", bufs=4, space="PSUM"
        ) as s1p:
            x_flat = x_d.rearrange("n h w c -> (n h w) c")
            dma_engines = [nc.sync, nc.gpsimd, nc.scalar]
            for t in range(M // 128):
                n, h0 = divmod(t, 8)
                h0 *= 4
                tf = s1.tile([128, 128], DT.float32)
                dma_engines[t % 3].dma_start(tf[:], x_flat[t * 128 : (t + 1) * 128, :])
                tb = s1.tile([128, 128], DT.bfloat16)
                nc.vector.tensor_copy(tb[:], tf[:])
                pp = s1p.tile([128, 128], DT.bfloat16)
                nc.tensor.transpose(pp[:], tb[:], ident[:])
                nc.vector.tensor_copy(
                    xa[:, n, 1 + h0 : 1 + h0 + 4, 1:33],
                    pp[:].rearrange("p (a b) -> p a b", a=4),
                )
        # ---- main loop
        # offsets handled by ScalarE (true |x-w|, +1 one-hot) vs VectorE
        # (min(x-w,0), -2 one-hot + sum(x) correction streams)
        act_set = set(range(9 - ACT_OFFS, 9))
        dve_offs = [oi for oi in range(9) if oi not in act_set]
        act_offs = [oi for oi in range(9) if oi in act_set]
        with tc.tile_pool(name="mp", bufs=1, space="PSUM") as mp, tc.tile_pool(
            name="udve", bufs=9
        ) as updve, tc.tile_pool(name="uact", bufs=6) as upact:
            P = mp.tile([128, M], DT.float32)

            # sum_c x streams (ones stationary), only for DVE-handled offsets;
            # the mixed offset covers only filter rows 0..95
            for k, oi in enumerate(dve_offs):
                i, j = OFFS[oi]
                for s in range(NCH):
                    n, h0 = divmod(s, 2)
                    nc.tensor.matmul(
                        P[:, s * CH : (s + 1) * CH],
                        ones96[:] if oi == mix_off else ones_s[:],
                        xa[:, n, i + h0 * 16 : i + h0 * 16 + 16, j : j + 32],
                        start=(k == 0),
                        stop=False,
                    )

            # filter group g covers filters {g, g+32, g+64, g+96}; strip js
            # computes filter g + 32*js as a 128x32 column-tiled matmul whose
            # PSUM partition range starts at 32*js, so PSUM row == filter.
            # Interleave ScalarE-class offsets among VectorE-class ones so the
            # PE queue roughly matches production-completion order.
            plan = list(dve_offs)
            for k, oi in enumerate(act_offs):
                plan.insert(3 + k * 4, oi)
            for g in range(FG):
                for oi in plan:
                    klass = "act" if oi in act_set else "dve"
                    if True:
                        i, j = OFFS[oi]
                        xin = xa[:, :, i : i + 32, :].rearrange(
                            "p n r c -> p n (r c)"
                        )
                        us = []
                        kls = []
                        for js in range(NSTRIP):
                            f = g + FG * js
                            wap = wt[:, oi, f : f + 1]
                            k2 = klass
                            if oi == mix_off and js == NSTRIP - 1:
                                k2 = "act"  # rebalance: ScalarE takes strip 3
                            pool = upact if k2 == "act" else updve
                            u = pool.tile([128, NL, 32 * PW], DT.bfloat16)
                            if k2 == "act":
                                nc.scalar.activation(
                                    u[:], xin, AF.Abs, bias=wap, scale=-1.0
                                )
                            else:
                                nc.vector.tensor_scalar(
                                    u[:], xin, wap, 0.0, AL.subtract, AL.min
                                )
                            us.append(u)
                            kls.append(k2)
                        last = g == FG - 1 and oi == plan[-1]
                        for s in range(NCH):
                            n, h0 = divmod(s, 2)
                            for js in range(NSTRIP):
                                zwin = zpos if kls[js] == "act" else zneg
                                stat = zwin[:, 127 - g : 159 - g]
                                u4 = us[js][:].rearrange(
                                    "p n (r c) -> p n r c", r=32
                                )
                                nc.tensor.matmul(
                                    P[32 * js : 32 * js + 32, s * CH : (s + 1) * CH],
                                    stat,
                                    u4[:, n, h0 * 16 : h0 * 16 + 16, j : j + 32],
                                    start=False,
                                    stop=last,
                                    tile_position=(0, 32 * js),
                                )

            # ---- drain: out = -P + (bias + sum w)
            sout = const.tile([128, M], DT.float32)
            for s in range(NCH):
                nc.scalar.activation(
                    sout[:, s * CH : (s + 1) * CH],
                    P[:, s * CH : (s + 1) * CH],
                    AF.Identity,
                    bias=bsum[:],
                    scale=-1.0,
                )

        # ---- transpose [f, m] -> [m, f] and store
        o_flat = o_d.rearrange("n h w f -> (n h w) f")
        with tc.tile_pool(name="tr", bufs=4) as trp, tc.tile_pool(
            name="trp", bufs=4, space="PSUM"
        ) as trpp:
            for t in range(M // 128):
                pt = trpp.tile([128, 128], DT.float32)
                nc.tensor.transpose(pt[:], sout[:, t * 128 : (t + 1) * 128], ident32[:])
                ot = trp.tile([128, 128], DT.float32)
                nc.vector.tensor_copy(ot[:], pt[:])
                nc.sync.dma_start(o_flat[t * 128 : (t + 1) * 128, :], ot[:])


_nc_cache = None


def _build():
    global _nc_cache
    if _nc_cache is None:
        nc = bacc.Bacc("TRN2", target_bir_lowering=False, debug=False, num_devices=N_CORES)
        x_d = nc.dram_tensor("inputs", [NL, H, W, C], DT.float32, kind="ExternalInput").ap()
        w_d = nc.dram_tensor("kernel", [3, 3, C, F], DT.float32, kind="ExternalInput").ap()
        b_d = nc.dram_tensor("bias", [1, F], DT.float32, kind="ExternalInput").ap()
        o_d = nc.dram_tensor("out", [NL, H, W, F], DT.float32, kind="ExternalOutput").ap()
        with tile.TileContext(nc) as tc:
            _body(tc, o_d, x_d, w_d, b_d)
        nc.compile()
        _nc_cache = nc
    return _nc_cache


def run(inputs, kernel, bias, **spmd_kwargs):
    nc = _build()
    shards = np.split(np.ascontiguousarray(inputs, dtype=np.float32), N_CORES, axis=0)
    kf = np.ascontiguousarray(kernel, dtype=np.float32)
    bf = np.ascontiguousarray(bias, dtype=np.float32).reshape(1, F)
    in_maps = [{"inputs": s, "kernel": kf, "bias": bf} for s in shards]
    res = run_bass_kernel_spmd(nc, in_maps, core_ids=list(range(N_CORES)), **spmd_kwargs)
    out = np.concatenate([r["out"] for r in res.results], axis=0)
    return out, res


def kernel(inputs, kernel, bias):
    out, _ = run(inputs, kernel, bias)
    return out


# revision 21
# speedup vs baseline: 1.7160x; 1.0026x over previous
"""Adder2D (L1-distance conv) Trainium2 kernel, data-parallel over batch on 8 cores.

out[n,h,w,f] = bias[f] - sum_{i,j,c} |x_pad[n, h+i, w+j, c] - kernel[i,j,c,f]|

Per-core shapes (batch 32 sharded 8 ways): x [4,32,32,128], kernel [3,3,128,128],
bias [128], out [4,32,32,128].

Decomposition used on-chip (avoids any abs op):
    |t| = t + 2*relu(-t)  with  t = x - w
 => sum_k |x-w| = sum_k x - sum_k w - 2*sum_k min(x-w, 0)
The channel dim C=128 sits on SBUF partitions; the 3x3 offsets are shifted views
of a zero-padded channels-first image. Per (filter, offset) one DVE
tensor_scalar(subtract, min) pass produces min(x-w,0) tiles in bf16 (4x mode);
the PE reduces over partitions with a (-2)-valued one-hot stationary,
accumulating into PSUM. sum_k x uses an all-ones stationary (filter-independent)
and sum_k w + bias folds into the per-filter bias applied when draining PSUM.
"""

import sys

if "/opt/trn_rl_repo" not in sys.path:
    sys.path.insert(0, "/opt/trn_rl_repo")

from contextlib import ExitStack

import numpy as np

import concourse.bass as bass  # noqa: F401
import concourse.tile as tile
from concourse import bacc, mybir
from concourse.bass_utils import run_bass_kernel_spmd
from concourse.masks import make_identity

AL = mybir.AluOpType
DT = mybir.dt
AF = mybir.ActivationFunctionType

N_CORES = 8
NL = 4  # images per core
H = W = 32
C = 128
F = 128
PH, PW = 34, 34  # padded rows / padded row pitch
M = NL * H * W  # 4096 output positions per core
CH = 512  # matmul moving chunk (one PSUM bank of fp32)
NCH = M // CH  # 8
NPAD = NL * PH * PW  # flat padded row length per partition (4896)

OFFS = [(i, j) for i in range(3) for j in range(3)]

# How many of the 9 offsets are computed on ScalarE (activation Abs with
# per-partition bias) instead of VectorE. ACT-handled offsets contribute
# sum|x-w| directly via a +1 one-hot stationary and skip the sum(x) stream.
ACT_OFFS = 2
# Column-tiling width for the PE reduction: 4 concurrent 128x32 matmuls.
NSTRIP = 4
FG = F // NSTRIP  # filter groups (32)


def _body(tc, o_d, x_d, w_d, b_d):
    nc = tc.nc
    with ExitStack() as ctx:
        const = ctx.enter_context(tc.tile_pool(name="const", bufs=1))

        ident = const.tile([128, 128], DT.bfloat16)
        make_identity(nc, ident[:])
        ident32 = const.tile([128, 128], DT.float32)
        make_identity(nc, ident32[:])

        # stationary tiles: window zneg[:, 127-f:255-f] is a one-hot column f
        zneg = const.tile([128, 255], DT.bfloat16)  # -2 at col 127
        nc.vector.memset(zneg[:], 0.0)
        nc.vector.memset(zneg[:, 127:128], -2.0)
        zpos = const.tile([128, 255], DT.bfloat16)  # +1 at col 127 (ACT offsets)
        nc.vector.memset(zpos[:], 0.0)
        nc.vector.memset(zpos[:, 127:128], 1.0)
        ones_s = const.tile([128, 128], DT.bfloat16)  # all-ones stationary
        nc.vector.memset(ones_s[:], 1.0)
        # ones over columns 0..96 only: sum(x) stream stationary for the
        # mixed offset whose strip 3 is handled by ScalarE
        ones96 = const.tile([128, 128], DT.bfloat16)
        nc.vector.memset(ones96[:], 0.0)
        nc.vector.memset(ones96[:, 0:96], 1.0)
        ones_col = const.tile([128, 1], DT.float32)
        nc.vector.memset(ones_col[:], 1.0)

        # padded channels-first input
        xa = const.tile([128, NL, PH, PW], DT.bfloat16)
        xa_flat = xa[:].rearrange("p n r c -> p (n r c)")
        nc.vector.memset(xa_flat, 0.0)

        # weights [c, off, f] fp32 and combined bias B[f] = bias[f] + sum_k w[k,f]
        wt = const.tile([128, 9, 128], DT.float32)
        nc.sync.dma_start(wt[:], w_d.rearrange("i j c f -> c (i j) f"))
        bias_row = const.tile([1, 128], DT.float32)
        nc.sync.dma_start(bias_row[:], b_d[:])

        # sum_k w only over VectorE-handled (offset, filter) units (ScalarE
        # units feed |x-w| directly and need no correction term). The mixed
        # offset MIX_OFF is VectorE for filters 0..95 and ScalarE for 96..127.
        dve_w = [oi for oi in range(9) if oi not in set(range(9 - ACT_OFFS, 9))]
        mix_off = dve_w[-1]
        wsum = const.tile([128, 128], DT.float32)
        nc.vector.tensor_tensor(wsum[:], wt[:, dve_w[0], :], wt[:, dve_w[1], :], AL.add)
        for o in dve_w[2:-1]:
            nc.vector.tensor_tensor(wsum[:], wsum[:], wt[:, o, :], AL.add)
        nc.vector.tensor_tensor(
            wsum[:, 0:96], wsum[:, 0:96], wt[:, mix_off, 0:96], AL.add
        )

        bsum = const.tile([128, 1], DT.float32)
        with tc.tile_pool(name="bp", bufs=1, space="PSUM") as bpp:
            bp = bpp.tile([128, 1], DT.float32)
            nc.tensor.matmul(bp[:], wsum[:], ones_col[:], start=True, stop=False)
            nc.tensor.matmul(
                bp[:], bias_row[:], ones_col[0:1, 0:1], start=False, stop=True
            )
            nc.vector.tensor_copy(bsum[:], bp[:])

        # ---- stage 1: DMA input, convert bf16, PE-transpose into padded buf
        with tc.tile_pool(name="s1", bufs=6) as s1, tc.tile_pool(
            name="s1p", bufs=6, space="PSUM"
        ) as s1p:
            x_flat = x_d.rearrange("n h w c -> (n h w) c")
            dma_engines = [nc.sync, nc.gpsimd, nc.scalar]
            for t in range(M // 128):
                n, h0 = divmod(t, 8)
                h0 *= 4
                tf = s1.tile([128, 128], DT.float32)
                dma_engines[t % 3].dma_start(tf[:], x_flat[t * 128 : (t + 1) * 128, :])
                tb = s1.tile([128, 128], DT.bfloat16)
                nc.vector.tensor_copy(tb[:], tf[:])
                pp = s1p.tile([128, 128], DT.bfloat16)
                nc.tensor.transpose(pp[:], tb[:], ident[:])
                nc.vector.tensor_copy(
                    xa[:, n, 1 + h0 : 1 + h0 + 4, 1:33],
                    pp[:].rearrange("p (a b) -> p a b", a=4),
                )
        # ---- main loop
        # offsets handled by ScalarE (true |x-w|, +1 one-hot) vs VectorE
        # (min(x-w,0), -2 one-hot + sum(x) correction streams)
        act_set = set(range(9 - ACT_OFFS, 9))
        dve_offs = [oi for oi in range(9) if oi not in act_set]
        act_offs = [oi for oi in range(9) if oi in act_set]
        with tc.tile_pool(name="mp", bufs=1, space="PSUM") as mp, tc.tile_pool(
            name="udve", bufs=10
        ) as updve, tc.tile_pool(name="uact", bufs=7) as upact:
            P = mp.tile([128, M], DT.float32)

            # sum_c x streams (ones stationary), only for DVE-handled offsets;
            # the mixed offset covers only filter rows 0..95
            for k, oi in enumerate(dve_offs):
                i, j = OFFS[oi]
                for s in range(NCH):
                    n, h0 = divmod(s, 2)
                    nc.tensor.matmul(
                        P[:, s * CH : (s + 1) * CH],
                        ones96[:] if oi == mix_off else ones_s[:],
                        xa[:, n, i + h0 * 16 : i + h0 * 16 + 16, j : j + 32],
                        start=(k == 0),
                        stop=False,
                    )

            # filter group g covers filters {g, g+32, g+64, g+96}; strip js
            # computes filter g + 32*js as a 128x32 column-tiled matmul whose
            # PSUM partition range starts at 32*js, so PSUM row == filter.
            # Interleave ScalarE-class offsets among VectorE-class ones so the
            # PE queue roughly matches production-completion order.
            plan = list(dve_offs)
            for k, oi in enumerate(act_offs):
                plan.insert(3 + k * 4, oi)
            for g in range(FG):
                for oi in plan:
                    klass = "act" if oi in act_set else "dve"
                    if True:
                        i, j = OFFS[oi]
                        xin = xa[:, :, i : i + 32, :].rearrange(
                            "p n r c -> p n (r c)"
                        )
                        us = []
                        kls = []
                        for js in range(NSTRIP):
                            f = g + FG * js
                            wap = wt[:, oi, f : f + 1]
                            k2 = klass
                            if oi == mix_off and js == NSTRIP - 1:
                                k2 = "act"  # rebalance: ScalarE takes strip 3
                            pool = upact if k2 == "act" else updve
                            u = pool.tile([128, NL, 32 * PW], DT.bfloat16)
                            if k2 == "act":
                                nc.scalar.activation(
                                    u[:], xin, AF.Abs, bias=wap, scale=-1.0
                                )
                            else:
                                nc.vector.tensor_scalar(
                                    u[:], xin, wap, 0.0, AL.subtract, AL.min
                                )
                            us.append(u)
                            kls.append(k2)
                        last = g == FG - 1 and oi == plan[-1]
                        for s in range(NCH):
                            n, h0 = divmod(s, 2)
                            for js in range(NSTRIP):
                                zwin = zpos if kls[js] == "act" else zneg
                                stat = zwin[:, 127 - g : 159 - g]
                                u4 = us[js][:].rearrange(
                                    "p n (r c) -> p n r c", r=32
                                )
                                nc.tensor.matmul(
                                    P[32 * js : 32 * js + 32, s * CH : (s + 1) * CH],
                                    stat,
                                    u4[:, n, h0 * 16 : h0 * 16 + 16, j : j + 32],
                                    start=False,
                                    stop=last,
                                    tile_position=(0, 32 * js),
                                )

            # ---- drain: out = -P + (bias + sum w)
            sout = const.tile([128, M], DT.float32)
            for s in range(NCH):
                nc.scalar.activation(
                    sout[:, s * CH : (s + 1) * CH],
                    P[:, s * CH : (s + 1) * CH],
                    AF.Identity,
                    bias=bsum[:],
                    scale=-1.0,
                )

        # ---- transpose [f, m] -> [m, f] and store
        o_flat = o_d.rearrange("n h w f -> (n h w) f")
        with tc.tile_pool(name="tr", bufs=4) as trp, tc.tile_pool(
            name="trp", bufs=4, space="PSUM"
        ) as trpp:
            for t in range(M // 128):
                pt = trpp.tile([128, 128], DT.float32)
                nc.tensor.transpose(pt[:], sout[:, t * 128 : (t + 1) * 128], ident32[:])
                ot = trp.tile([128, 128], DT.float32)
                nc.vector.tensor_copy(ot[:], pt[:])
                nc.sync.dma_start(o_flat[t * 128 : (t + 1) * 128, :], ot[:])


_nc_cache = None


def _build():
    global _nc_cache
    if _nc_cache is None:
        nc = bacc.Bacc("TRN2", target_bir_lowering=False, debug=False, num_devices=N_CORES)
        x_d = nc.dram_tensor("inputs", [NL, H, W, C], DT.float32, kind="ExternalInput").ap()
        w_d = nc.dram_tensor("kernel", [3, 3, C, F], DT.float32, kind="ExternalInput").ap()
        b_d = nc.dram_tensor("bias", [1, F], DT.float32, kind="ExternalInput").ap()
        o_d = nc.dram_tensor("out", [NL, H, W, F], DT.float32, kind="ExternalOutput").ap()
        with tile.TileContext(nc) as tc:
            _body(tc, o_d, x_d, w_d, b_d)
        nc.compile()
        _nc_cache = nc
    return _nc_cache


def run(inputs, kernel, bias, **spmd_kwargs):
    nc = _build()
    shards = np.split(np.ascontiguousarray(inputs, dtype=np.float32), N_CORES, axis=0)
    kf = np.ascontiguousarray(kernel, dtype=np.float32)
    bf = np.ascontiguousarray(bias, dtype=np.float32).reshape(1, F)
    in_maps = [{"inputs": s, "kernel": kf, "bias": bf} for s in shards]
    res = run_bass_kernel_spmd(nc, in_maps, core_ids=list(range(N_CORES)), **spmd_kwargs)
    out = np.concatenate([r["out"] for r in res.results], axis=0)
    return out, res


def kernel(inputs, kernel, bias):
    out, _ = run(inputs, kernel, bias)
    return out


# revision 22
# speedup vs baseline: 1.7200x; 1.0023x over previous
"""Adder2D (L1-distance conv) Trainium2 kernel, data-parallel over batch on 8 cores.

out[n,h,w,f] = bias[f] - sum_{i,j,c} |x_pad[n, h+i, w+j, c] - kernel[i,j,c,f]|

Per-core shapes (batch 32 sharded 8 ways): x [4,32,32,128], kernel [3,3,128,128],
bias [128], out [4,32,32,128].

Decomposition used on-chip (avoids any abs op):
    |t| = t + 2*relu(-t)  with  t = x - w
 => sum_k |x-w| = sum_k x - sum_k w - 2*sum_k min(x-w, 0)
The channel dim C=128 sits on SBUF partitions; the 3x3 offsets are shifted views
of a zero-padded channels-first image. Per (filter, offset) one DVE
tensor_scalar(subtract, min) pass produces min(x-w,0) tiles in bf16 (4x mode);
the PE reduces over partitions with a (-2)-valued one-hot stationary,
accumulating into PSUM. sum_k x uses an all-ones stationary (filter-independent)
and sum_k w + bias folds into the per-filter bias applied when draining PSUM.
"""

import sys

if "/opt/trn_rl_repo" not in sys.path:
    sys.path.insert(0, "/opt/trn_rl_repo")

from contextlib import ExitStack

import numpy as np

import concourse.bass as bass  # noqa: F401
import concourse.tile as tile
from concourse import bacc, mybir
from concourse.bass_utils import run_bass_kernel_spmd
from concourse.masks import make_identity

AL = mybir.AluOpType
DT = mybir.dt
AF = mybir.ActivationFunctionType

N_CORES = 8
NL = 4  # images per core
H = W = 32
C = 128
F = 128
PH, PW = 34, 34  # padded rows / padded row pitch
M = NL * H * W  # 4096 output positions per core
CH = 512  # matmul moving chunk (one PSUM bank of fp32)
NCH = M // CH  # 8
NPAD = NL * PH * PW  # flat padded row length per partition (4896)

OFFS = [(i, j) for i in range(3) for j in range(3)]

# How many of the 9 offsets are computed on ScalarE (activation Abs with
# per-partition bias) instead of VectorE. ACT-handled offsets contribute
# sum|x-w| directly via a +1 one-hot stationary and skip the sum(x) stream.
ACT_OFFS = 2
# Column-tiling width for the PE reduction: 4 concurrent 128x32 matmuls.
NSTRIP = 4
FG = F // NSTRIP  # filter groups (32)


def _body(tc, o_d, x_d, w_d, b_d):
    nc = tc.nc
    with ExitStack() as ctx:
        const = ctx.enter_context(tc.tile_pool(name="const", bufs=1))

        ident = const.tile([128, 128], DT.bfloat16)
        make_identity(nc, ident[:])
        ident32 = const.tile([128, 128], DT.float32)
        make_identity(nc, ident32[:])

        # stationary tiles: window zneg[:, 127-f:255-f] is a one-hot column f
        zneg = const.tile([128, 255], DT.bfloat16)  # -2 at col 127
        nc.vector.memset(zneg[:], 0.0)
        nc.vector.memset(zneg[:, 127:128], -2.0)
        zpos = const.tile([128, 255], DT.bfloat16)  # +1 at col 127 (ACT offsets)
        nc.vector.memset(zpos[:], 0.0)
        nc.vector.memset(zpos[:, 127:128], 1.0)
        ones_s = const.tile([128, 128], DT.bfloat16)  # all-ones stationary
        nc.vector.memset(ones_s[:], 1.0)
        # ones over columns 0..96 only: sum(x) stream stationary for the
        # mixed offset whose strip 3 is handled by ScalarE
        ones96 = const.tile([128, 128], DT.bfloat16)
        nc.vector.memset(ones96[:], 0.0)
        nc.vector.memset(ones96[:, 0:96], 1.0)
        ones_col = const.tile([128, 1], DT.float32)
        nc.vector.memset(ones_col[:], 1.0)

        # padded channels-first input
        xa = const.tile([128, NL, PH, PW], DT.bfloat16)
        xa_flat = xa[:].rearrange("p n r c -> p (n r c)")
        nc.vector.memset(xa_flat, 0.0)

        # weights [c, off, f] fp32 and combined bias B[f] = bias[f] + sum_k w[k,f]
        wt = const.tile([128, 9, 128], DT.float32)
        nc.sync.dma_start(wt[:], w_d.rearrange("i j c f -> c (i j) f"))
        bias_row = const.tile([1, 128], DT.float32)
        nc.sync.dma_start(bias_row[:], b_d[:])

        # sum_k w only over VectorE-handled (offset, filter) units (ScalarE
        # units feed |x-w| directly and need no correction term). The mixed
        # offset MIX_OFF is VectorE for filters 0..95 and ScalarE for 96..127.
        dve_w = [oi for oi in range(9) if oi not in set(range(9 - ACT_OFFS, 9))]
        mix_off = dve_w[-1]
        wsum = const.tile([128, 128], DT.float32)
        nc.vector.tensor_tensor(wsum[:], wt[:, dve_w[0], :], wt[:, dve_w[1], :], AL.add)
        for o in dve_w[2:-1]:
            nc.vector.tensor_tensor(wsum[:], wsum[:], wt[:, o, :], AL.add)
        nc.vector.tensor_tensor(
            wsum[:, 0:96], wsum[:, 0:96], wt[:, mix_off, 0:96], AL.add
        )

        bsum = const.tile([128, 1], DT.float32)
        with tc.tile_pool(name="bp", bufs=1, space="PSUM") as bpp:
            bp = bpp.tile([128, 1], DT.float32)
            nc.tensor.matmul(bp[:], wsum[:], ones_col[:], start=True, stop=False)
            nc.tensor.matmul(
                bp[:], bias_row[:], ones_col[0:1, 0:1], start=False, stop=True
            )
            nc.vector.tensor_copy(bsum[:], bp[:])

        # ---- stage 1: DMA input, convert bf16, PE-transpose into padded buf
        with tc.tile_pool(name="s1", bufs=6) as s1, tc.tile_pool(
            name="s1p", bufs=6, space="PSUM"
        ) as s1p:
            x_flat = x_d.rearrange("n h w c -> (n h w) c")
            dma_engines = [nc.sync, nc.gpsimd, nc.scalar]
            for t in range(M // 128):
                n, h0 = divmod(t, 8)
                h0 *= 4
                tf = s1.tile([128, 128], DT.float32)
                dma_engines[t % 3].dma_start(tf[:], x_flat[t * 128 : (t + 1) * 128, :])
                tb = s1.tile([128, 128], DT.bfloat16)
                nc.vector.tensor_copy(tb[:], tf[:])
                pp = s1p.tile([128, 128], DT.bfloat16)
                nc.tensor.transpose(pp[:], tb[:], ident[:])
                nc.vector.tensor_copy(
                    xa[:, n, 1 + h0 : 1 + h0 + 4, 1:33],
                    pp[:].rearrange("p (a b) -> p a b", a=4),
                )
        # ---- main loop
        # offsets handled by ScalarE (true |x-w|, +1 one-hot) vs VectorE
        # (min(x-w,0), -2 one-hot + sum(x) correction streams)
        act_set = set(range(9 - ACT_OFFS, 9))
        dve_offs = [oi for oi in range(9) if oi not in act_set]
        act_offs = [oi for oi in range(9) if oi in act_set]
        with tc.tile_pool(name="mp", bufs=1, space="PSUM") as mp, tc.tile_pool(
            name="udve", bufs=9
        ) as updve, tc.tile_pool(name="uact", bufs=9) as upact:
            P = mp.tile([128, M], DT.float32)

            # sum_c x streams (ones stationary), only for DVE-handled offsets;
            # the mixed offset covers only filter rows 0..95
            for k, oi in enumerate(dve_offs):
                i, j = OFFS[oi]
                for s in range(NCH):
                    n, h0 = divmod(s, 2)
                    nc.tensor.matmul(
                        P[:, s * CH : (s + 1) * CH],
                        ones96[:] if oi == mix_off else ones_s[:],
                        xa[:, n, i + h0 * 16 : i + h0 * 16 + 16, j : j + 32],
                        start=(k == 0),
                        stop=False,
                    )

            # filter group g covers filters {g, g+32, g+64, g+96}; strip js
            # computes filter g + 32*js as a 128x32 column-tiled matmul whose
            # PSUM partition range starts at 32*js, so PSUM row == filter.
            # Interleave ScalarE-class offsets among VectorE-class ones so the
            # PE queue roughly matches production-completion order.
            plan = list(dve_offs)
            for k, oi in enumerate(act_offs):
                plan.insert(3 + k * 4, oi)
            for g in range(FG):
                for oi in plan:
                    klass = "act" if oi in act_set else "dve"
                    if True:
                        i, j = OFFS[oi]
                        xin = xa[:, :, i : i + 32, :].rearrange(
                            "p n r c -> p n (r c)"
                        )
                        us = []
                        kls = []
                        for js in range(NSTRIP):
                            f = g + FG * js
                            wap = wt[:, oi, f : f + 1]
                            k2 = klass
                            if oi == mix_off and js == NSTRIP - 1:
                                k2 = "act"  # rebalance: ScalarE takes strip 3
                            pool = upact if k2 == "act" else updve
                            u = pool.tile([128, NL, 32 * PW], DT.bfloat16)
                            if k2 == "act":
                                nc.scalar.activation(
                                    u[:], xin, AF.Abs, bias=wap, scale=-1.0
                                )
                            else:
                                nc.vector.tensor_scalar(
                                    u[:], xin, wap, 0.0, AL.subtract, AL.min
                                )
                            us.append(u)
                            kls.append(k2)
                        last = g == FG - 1 and oi == plan[-1]
                        for s in range(NCH):
                            n, h0 = divmod(s, 2)
                            for js in range(NSTRIP):
                                zwin = zpos if kls[js] == "act" else zneg
                                stat = zwin[:, 127 - g : 159 - g]
                                u4 = us[js][:].rearrange(
                                    "p n (r c) -> p n r c", r=32
                                )
                                nc.tensor.matmul(
                                    P[32 * js : 32 * js + 32, s * CH : (s + 1) * CH],
                                    stat,
                                    u4[:, n, h0 * 16 : h0 * 16 + 16, j : j + 32],
                                    start=False,
                                    stop=last,
                                    tile_position=(0, 32 * js),
                                )

            # ---- drain: out = -P + (bias + sum w)
            sout = const.tile([128, M], DT.float32)
            for s in range(NCH):
                nc.scalar.activation(
                    sout[:, s * CH : (s + 1) * CH],
                    P[:, s * CH : (s + 1) * CH],
                    AF.Identity,
                    bias=bsum[:],
                    scale=-1.0,
                )

        # ---- transpose [f, m] -> [m, f] and store
        o_flat = o_d.rearrange("n h w f -> (n h w) f")
        with tc.tile_pool(name="tr", bufs=4) as trp, tc.tile_pool(
            name="trp", bufs=4, space="PSUM"
        ) as trpp:
            for t in range(M // 128):
                pt = trpp.tile([128, 128], DT.float32)
                nc.tensor.transpose(pt[:], sout[:, t * 128 : (t + 1) * 128], ident32[:])
                ot = trp.tile([128, 128], DT.float32)
                nc.vector.tensor_copy(ot[:], pt[:])
                nc.sync.dma_start(o_flat[t * 128 : (t + 1) * 128, :], ot[:])


_nc_cache = None


def _build():
    global _nc_cache
    if _nc_cache is None:
        nc = bacc.Bacc("TRN2", target_bir_lowering=False, debug=False, num_devices=N_CORES)
        x_d = nc.dram_tensor("inputs", [NL, H, W, C], DT.float32, kind="ExternalInput").ap()
        w_d = nc.dram_tensor("kernel", [3, 3, C, F], DT.float32, kind="ExternalInput").ap()
        b_d = nc.dram_tensor("bias", [1, F], DT.float32, kind="ExternalInput").ap()
        o_d = nc.dram_tensor("out", [NL, H, W, F], DT.float32, kind="ExternalOutput").ap()
        with tile.TileContext(nc) as tc:
            _body(tc, o_d, x_d, w_d, b_d)
        nc.compile()
        _nc_cache = nc
    return _nc_cache


def run(inputs, kernel, bias, **spmd_kwargs):
    nc = _build()
    shards = np.split(np.ascontiguousarray(inputs, dtype=np.float32), N_CORES, axis=0)
    kf = np.ascontiguousarray(kernel, dtype=np.float32)
    bf = np.ascontiguousarray(bias, dtype=np.float32).reshape(1, F)
    in_maps = [{"inputs": s, "kernel": kf, "bias": bf} for s in shards]
    res = run_bass_kernel_spmd(nc, in_maps, core_ids=list(range(N_CORES)), **spmd_kwargs)
    out = np.concatenate([r["out"] for r in res.results], axis=0)
    return out, res


def kernel(inputs, kernel, bias):
    out, _ = run(inputs, kernel, bias)
    return out


# revision 24
# speedup vs baseline: 1.7266x; 1.0038x over previous
"""Adder2D (L1-distance conv) Trainium2 kernel, data-parallel over batch on 8 cores.

out[n,h,w,f] = bias[f] - sum_{i,j,c} |x_pad[n, h+i, w+j, c] - kernel[i,j,c,f]|

Per-core shapes (batch 32 sharded 8 ways): x [4,32,32,128], kernel [3,3,128,128],
bias [128], out [4,32,32,128].

Decomposition used on-chip (avoids any abs op):
    |t| = t + 2*relu(-t)  with  t = x - w
 => sum_k |x-w| = sum_k x - sum_k w - 2*sum_k min(x-w, 0)
The channel dim C=128 sits on SBUF partitions; the 3x3 offsets are shifted views
of a zero-padded channels-first image. Per (filter, offset) one DVE
tensor_scalar(subtract, min) pass produces min(x-w,0) tiles in bf16 (4x mode);
the PE reduces over partitions with a (-2)-valued one-hot stationary,
accumulating into PSUM. sum_k x uses an all-ones stationary (filter-independent)
and sum_k w + bias folds into the per-filter bias applied when draining PSUM.
"""

import sys

if "/opt/trn_rl_repo" not in sys.path:
    sys.path.insert(0, "/opt/trn_rl_repo")

from contextlib import ExitStack

import numpy as np

import concourse.bass as bass  # noqa: F401
import concourse.tile as tile
from concourse import bacc, mybir
from concourse.bass_utils import run_bass_kernel_spmd
from concourse.masks import make_identity

AL = mybir.AluOpType
DT = mybir.dt
AF = mybir.ActivationFunctionType

N_CORES = 8
NL = 4  # images per core
H = W = 32
C = 128
F = 128
PH, PW = 34, 34  # padded rows / padded row pitch
M = NL * H * W  # 4096 output positions per core
CH = 512  # matmul moving chunk (one PSUM bank of fp32)
NCH = M // CH  # 8
NPAD = NL * PH * PW  # flat padded row length per partition (4896)

OFFS = [(i, j) for i in range(3) for j in range(3)]

# How many of the 9 offsets are computed on ScalarE (activation Abs with
# per-partition bias) instead of VectorE. ACT-handled offsets contribute
# sum|x-w| directly via a +1 one-hot stationary and skip the sum(x) stream.
ACT_OFFS = 2
# Column-tiling width for the PE reduction: 4 concurrent 128x32 matmuls.
NSTRIP = 4
FG = F // NSTRIP  # filter groups (32)


def _body(tc, o_d, x_d, w_d, b_d):
    nc = tc.nc
    with ExitStack() as ctx:
        const = ctx.enter_context(tc.tile_pool(name="const", bufs=1))

        ident = const.tile([128, 128], DT.bfloat16)
        make_identity(nc, ident[:])
        ident32 = const.tile([128, 128], DT.float32)
        make_identity(nc, ident32[:])

        # stationary tiles: window zneg[:, 127-f:255-f] is a one-hot column f
        zneg = const.tile([128, 255], DT.bfloat16)  # -2 at col 127
        nc.vector.memset(zneg[:], 0.0)
        nc.vector.memset(zneg[:, 127:128], -2.0)
        zpos = const.tile([128, 255], DT.bfloat16)  # +1 at col 127 (ACT offsets)
        nc.vector.memset(zpos[:], 0.0)
        nc.vector.memset(zpos[:, 127:128], 1.0)
        ones_s = const.tile([128, 128], DT.bfloat16)  # all-ones stationary
        nc.vector.memset(ones_s[:], 1.0)
        # ones over columns 0..96 only: sum(x) stream stationary for the
        # mixed offset whose strip 3 is handled by ScalarE
        ones96 = const.tile([128, 128], DT.bfloat16)
        nc.vector.memset(ones96[:], 0.0)
        nc.vector.memset(ones96[:, 0:96], 1.0)
        ones_col = const.tile([128, 1], DT.float32)
        nc.vector.memset(ones_col[:], 1.0)

        # padded channels-first input
        xa = const.tile([128, NL, PH, PW], DT.bfloat16)
        xa_flat = xa[:].rearrange("p n r c -> p (n r c)")
        nc.vector.memset(xa_flat, 0.0)

        # weights [c, off, f] fp32 and combined bias B[f] = bias[f] + sum_k w[k,f]
        wt = const.tile([128, 9, 128], DT.float32)
        nc.sync.dma_start(wt[:], w_d.rearrange("i j c f -> c (i j) f"))
        bias_row = const.tile([1, 128], DT.float32)
        nc.sync.dma_start(bias_row[:], b_d[:])

        # sum_k w only over VectorE-handled (offset, filter) units (ScalarE
        # units feed |x-w| directly and need no correction term). The mixed
        # offset MIX_OFF is VectorE for filters 0..95 and ScalarE for 96..127.
        dve_w = [oi for oi in range(9) if oi not in set(range(9 - ACT_OFFS, 9))]
        mix_off = dve_w[-1]
        wsum = const.tile([128, 128], DT.float32)
        nc.vector.tensor_tensor(wsum[:], wt[:, dve_w[0], :], wt[:, dve_w[1], :], AL.add)
        for o in dve_w[2:-1]:
            nc.vector.tensor_tensor(wsum[:], wsum[:], wt[:, o, :], AL.add)
        nc.vector.tensor_tensor(
            wsum[:, 0:96], wsum[:, 0:96], wt[:, mix_off, 0:96], AL.add
        )

        bsum = const.tile([128, 1], DT.float32)
        with tc.tile_pool(name="bp", bufs=1, space="PSUM") as bpp:
            bp = bpp.tile([128, 1], DT.float32)
            nc.tensor.matmul(bp[:], wsum[:], ones_col[:], start=True, stop=False)
            nc.tensor.matmul(
                bp[:], bias_row[:], ones_col[0:1, 0:1], start=False, stop=True
            )
            nc.vector.tensor_copy(bsum[:], bp[:])

        # ---- stage 1: DMA input, convert bf16, PE-transpose into padded buf
        with tc.tile_pool(name="s1", bufs=6) as s1, tc.tile_pool(
            name="s1p", bufs=6, space="PSUM"
        ) as s1p:
            x_flat = x_d.rearrange("n h w c -> (n h w) c")
            dma_engines = [nc.sync, nc.gpsimd, nc.scalar]
            for t in range(M // 128):
                n, h0 = divmod(t, 8)
                h0 *= 4
                tf = s1.tile([128, 128], DT.float32)
                dma_engines[t % 3].dma_start(tf[:], x_flat[t * 128 : (t + 1) * 128, :])
                tb = s1.tile([128, 128], DT.bfloat16)
                nc.scalar.copy(tb[:], tf[:])
                pp = s1p.tile([128, 128], DT.bfloat16)
                nc.tensor.transpose(pp[:], tb[:], ident[:])
                nc.vector.tensor_copy(
                    xa[:, n, 1 + h0 : 1 + h0 + 4, 1:33],
                    pp[:].rearrange("p (a b) -> p a b", a=4),
                )
        # ---- main loop
        # offsets handled by ScalarE (true |x-w|, +1 one-hot) vs VectorE
        # (min(x-w,0), -2 one-hot + sum(x) correction streams)
        act_set = set(range(9 - ACT_OFFS, 9))
        dve_offs = [oi for oi in range(9) if oi not in act_set]
        act_offs = [oi for oi in range(9) if oi in act_set]
        with tc.tile_pool(name="mp", bufs=1, space="PSUM") as mp, tc.tile_pool(
            name="udve", bufs=9
        ) as updve, tc.tile_pool(name="uact", bufs=9) as upact:
            P = mp.tile([128, M], DT.float32)

            # sum_c x streams (ones stationary), only for DVE-handled offsets;
            # the mixed offset covers only filter rows 0..95
            for k, oi in enumerate(dve_offs):
                i, j = OFFS[oi]
                for s in range(NCH):
                    n, h0 = divmod(s, 2)
                    nc.tensor.matmul(
                        P[:, s * CH : (s + 1) * CH],
                        ones96[:] if oi == mix_off else ones_s[:],
                        xa[:, n, i + h0 * 16 : i + h0 * 16 + 16, j : j + 32],
                        start=(k == 0),
                        stop=False,
                    )

            # filter group g covers filters {g, g+32, g+64, g+96}; strip js
            # computes filter g + 32*js as a 128x32 column-tiled matmul whose
            # PSUM partition range starts at 32*js, so PSUM row == filter.
            # Interleave ScalarE-class offsets among VectorE-class ones so the
            # PE queue roughly matches production-completion order.
            plan = list(dve_offs)
            for k, oi in enumerate(act_offs):
                plan.insert(3 + k * 4, oi)
            for g in range(FG):
                for oi in plan:
                    klass = "act" if oi in act_set else "dve"
                    if True:
                        i, j = OFFS[oi]
                        xin = xa[:, :, i : i + 32, :].rearrange(
                            "p n r c -> p n (r c)"
                        )
                        us = []
                        kls = []
                        for js in range(NSTRIP):
                            f = g + FG * js
                            wap = wt[:, oi, f : f + 1]
                            k2 = klass
                            if oi == mix_off and js == NSTRIP - 1:
                                k2 = "act"  # rebalance: ScalarE takes strip 3
                            pool = upact if k2 == "act" else updve
                            u = pool.tile([128, NL, 32 * PW], DT.bfloat16)
                            if k2 == "act":
                                nc.scalar.activation(
                                    u[:], xin, AF.Abs, bias=wap, scale=-1.0
                                )
                            else:
                                nc.vector.tensor_scalar(
                                    u[:], xin, wap, 0.0, AL.subtract, AL.min
                                )
                            us.append(u)
                            kls.append(k2)
                        last = g == FG - 1 and oi == plan[-1]
                        for s in range(NCH):
                            n, h0 = divmod(s, 2)
                            for js in range(NSTRIP):
                                zwin = zpos if kls[js] == "act" else zneg
                                stat = zwin[:, 127 - g : 159 - g]
                                u4 = us[js][:].rearrange(
                                    "p n (r c) -> p n r c", r=32
                                )
                                nc.tensor.matmul(
                                    P[32 * js : 32 * js + 32, s * CH : (s + 1) * CH],
                                    stat,
                                    u4[:, n, h0 * 16 : h0 * 16 + 16, j : j + 32],
                                    start=False,
                                    stop=last,
                                    tile_position=(0, 32 * js),
                                )

            # ---- drain: out = -P + (bias + sum w)
            sout = const.tile([128, M], DT.float32)
            for s in range(NCH):
                nc.scalar.activation(
                    sout[:, s * CH : (s + 1) * CH],
                    P[:, s * CH : (s + 1) * CH],
                    AF.Identity,
                    bias=bsum[:],
                    scale=-1.0,
                )

        # ---- transpose [f, m] -> [m, f] and store
        o_flat = o_d.rearrange("n h w f -> (n h w) f")
        with tc.tile_pool(name="tr", bufs=6) as trp, tc.tile_pool(
            name="trp", bufs=6, space="PSUM"
        ) as trpp:
            dma_engines = [nc.sync, nc.gpsimd, nc.scalar]
            for t in range(M // 128):
                pt = trpp.tile([128, 128], DT.float32)
                nc.tensor.transpose(pt[:], sout[:, t * 128 : (t + 1) * 128], ident32[:])
                ot = trp.tile([128, 128], DT.float32)
                nc.vector.tensor_copy(ot[:], pt[:])
                dma_engines[t % 3].dma_start(o_flat[t * 128 : (t + 1) * 128, :], ot[:])


_nc_cache = None


def _build():
    global _nc_cache
    if _nc_cache is None:
        nc = bacc.Bacc("TRN2", target_bir_lowering=False, debug=False, num_devices=N_CORES)
        x_d = nc.dram_tensor("inputs", [NL, H, W, C], DT.float32, kind="ExternalInput").ap()
        w_d = nc.dram_tensor("kernel", [3, 3, C, F], DT.float32, kind="ExternalInput").ap()
        b_d = nc.dram_tensor("bias", [1, F], DT.float32, kind="ExternalInput").ap()
        o_d = nc.dram_tensor("out", [NL, H, W, F], DT.float32, kind="ExternalOutput").ap()
        with tile.TileContext(nc) as tc:
            _body(tc, o_d, x_d, w_d, b_d)
        nc.compile()
        _nc_cache = nc
    return _nc_cache


def run(inputs, kernel, bias, **spmd_kwargs):
    nc = _build()
    shards = np.split(np.ascontiguousarray(inputs, dtype=np.float32), N_CORES, axis=0)
    kf = np.ascontiguousarray(kernel, dtype=np.float32)
    bf = np.ascontiguousarray(bias, dtype=np.float32).reshape(1, F)
    in_maps = [{"inputs": s, "kernel": kf, "bias": bf} for s in shards]
    res = run_bass_kernel_spmd(nc, in_maps, core_ids=list(range(N_CORES)), **spmd_kwargs)
    out = np.concatenate([r["out"] for r in res.results], axis=0)
    return out, res


def kernel(inputs, kernel, bias):
    out, _ = run(inputs, kernel, bias)
    return out


# revision 27
# speedup vs baseline: 1.7327x; 1.0035x over previous
"""Adder2D (L1-distance conv) Trainium2 kernel, data-parallel over batch on 8 cores.

out[n,h,w,f] = bias[f] - sum_{i,j,c} |x_pad[n, h+i, w+j, c] - kernel[i,j,c,f]|

Per-core shapes (batch 32 sharded 8 ways): x [4,32,32,128], kernel [3,3,128,128],
bias [128], out [4,32,32,128].

Decomposition used on-chip (the DVE has no one-pass abs-diff op):
    |t| = t + 2*relu(-t)  with  t = x - w
 => sum_k |x-w| = sum_k x - sum_k w - 2*sum_k min(x-w, 0)
The channel dim C=128 sits on SBUF partitions; the 3x3 offsets are shifted views
of a zero-padded channels-first image. Per (filter, offset) unit, one engine
pass produces a u tile in bf16 over whole contiguous padded rows:
  - VectorE units: tensor_scalar(subtract, min) at 4x mode -> min(x-w, 0),
    reduced over partitions by the PE with a (-2)-valued one-hot stationary;
  - ScalarE units (2 of 9 offsets + one strip of a third, to balance the two
    engines): activation(Abs, bias=-w) -> |x-w|, reduced with a +1 one-hot.
The PE runs 4 column-tiled 128x32 matmuls concurrently (filter groups
{g, g+32, g+64, g+96}, PSUM row == filter index), accumulating all offsets and
filters into a [128, 4096] PSUM tile. sum_k x uses an all-ones stationary
(filter-independent); sum_k w + bias folds into the per-filter bias applied on
the ScalarE pass that drains PSUM (out = -P + B). A PE-transpose pass then
converts [f, m] -> [m, f] for contiguous output DMA.

Measured on 8 axon trn2 cores: ~1.16 ms NEFF exec, rel err ~9e-5 vs the fp32
reference (VectorE and ScalarE both >95% busy; producer-bound).
"""

import sys

if "/opt/trn_rl_repo" not in sys.path:
    sys.path.insert(0, "/opt/trn_rl_repo")

from contextlib import ExitStack

import numpy as np

import concourse.bass as bass  # noqa: F401
import concourse.tile as tile
from concourse import bacc, mybir
from concourse.bass_utils import run_bass_kernel_spmd
from concourse.masks import make_identity

AL = mybir.AluOpType
DT = mybir.dt
AF = mybir.ActivationFunctionType

N_CORES = 8
NL = 4  # images per core
H = W = 32
C = 128
F = 128
PH, PW = 34, 34  # padded rows / padded row pitch
M = NL * H * W  # 4096 output positions per core
CH = 512  # matmul moving chunk (one PSUM bank of fp32)
NCH = M // CH  # 8
NPAD = NL * PH * PW  # flat padded length per partition (4624)

OFFS = [(i, j) for i in range(3) for j in range(3)]

# How many of the 9 offsets are computed on ScalarE (activation Abs with
# per-partition bias) instead of VectorE. ACT-handled offsets contribute
# sum|x-w| directly via a +1 one-hot stationary and skip the sum(x) stream.
ACT_OFFS = 2
# Column-tiling width for the PE reduction: 4 concurrent 128x32 matmuls.
NSTRIP = 4
FG = F // NSTRIP  # filter groups (32)


def _body(tc, o_d, x_d, w_d, b_d):
    nc = tc.nc
    with ExitStack() as ctx:
        const = ctx.enter_context(tc.tile_pool(name="const", bufs=1))

        ident = const.tile([128, 128], DT.bfloat16)
        make_identity(nc, ident[:])
        ident32 = const.tile([128, 128], DT.float32)
        make_identity(nc, ident32[:])

        # stationary tiles: window zneg[:, 127-f:255-f] is a one-hot column f
        zneg = const.tile([128, 255], DT.bfloat16)  # -2 at col 127
        nc.vector.memset(zneg[:], 0.0)
        nc.vector.memset(zneg[:, 127:128], -2.0)
        zpos = const.tile([128, 255], DT.bfloat16)  # +1 at col 127 (ACT offsets)
        nc.vector.memset(zpos[:], 0.0)
        nc.vector.memset(zpos[:, 127:128], 1.0)
        ones_s = const.tile([128, 128], DT.bfloat16)  # all-ones stationary
        nc.vector.memset(ones_s[:], 1.0)
        # ones over columns 0..96 only: sum(x) stream stationary for the
        # mixed offset whose strip 3 is handled by ScalarE
        ones96 = const.tile([128, 128], DT.bfloat16)
        nc.vector.memset(ones96[:], 0.0)
        nc.vector.memset(ones96[:, 0:96], 1.0)
        ones_col = const.tile([128, 1], DT.float32)
        nc.vector.memset(ones_col[:], 1.0)

        # padded channels-first input
        xa = const.tile([128, NL, PH, PW], DT.bfloat16)
        xa_flat = xa[:].rearrange("p n r c -> p (n r c)")
        nc.vector.memset(xa_flat, 0.0)

        # weights [c, off, f] fp32 and combined bias B[f] = bias[f] + sum_k w[k,f]
        wt = const.tile([128, 9, 128], DT.float32)
        nc.sync.dma_start(wt[:], w_d.rearrange("i j c f -> c (i j) f"))
        bias_row = const.tile([1, 128], DT.float32)
        nc.sync.dma_start(bias_row[:], b_d[:])

        # sum_k w only over VectorE-handled (offset, filter) units (ScalarE
        # units feed |x-w| directly and need no correction term). The mixed
        # offset MIX_OFF is VectorE for filters 0..95 and ScalarE for 96..127.
        dve_w = [oi for oi in range(9) if oi not in set(range(9 - ACT_OFFS, 9))]
        mix_off = dve_w[-1]
        wsum = const.tile([128, 128], DT.float32)
        nc.vector.tensor_tensor(wsum[:], wt[:, dve_w[0], :], wt[:, dve_w[1], :], AL.add)
        for o in dve_w[2:-1]:
            nc.vector.tensor_tensor(wsum[:], wsum[:], wt[:, o, :], AL.add)
        nc.vector.tensor_tensor(
            wsum[:, 0:96], wsum[:, 0:96], wt[:, mix_off, 0:96], AL.add
        )

        bsum = const.tile([128, 1], DT.float32)
        with tc.tile_pool(name="bp", bufs=1, space="PSUM") as bpp:
            bp = bpp.tile([128, 1], DT.float32)
            nc.tensor.matmul(bp[:], wsum[:], ones_col[:], start=True, stop=False)
            nc.tensor.matmul(
                bp[:], bias_row[:], ones_col[0:1, 0:1], start=False, stop=True
            )
            nc.vector.tensor_copy(bsum[:], bp[:])

        # ---- stage 1: DMA input (4 bulk transfers), convert bf16,
        # PE-transpose into the padded channels-first buffer
        with tc.tile_pool(name="s1", bufs=6) as s1, tc.tile_pool(
            name="s1p", bufs=6, space="PSUM"
        ) as s1p:
            # m = b*128 + p: staging[k][p, b, c] holds m-tiles b of chunk k
            x_blk = x_d.rearrange("n h w c -> (n h w) c").rearrange(
                "(b p) c -> p b c", p=128
            )
            dma_engines = [nc.sync, nc.gpsimd, nc.scalar]
            stgs = []
            for k in range(4):
                stg = s1.tile([128, 8, 128], DT.float32, tag=f"stg{k}")
                dma_engines[k % 3].dma_start(stg[:], x_blk[:, k * 8 : (k + 1) * 8, :])
                stgs.append(stg)
            for t in range(M // 128):
                n, h0 = divmod(t, 8)
                h0 *= 4
                tb = s1.tile([128, 128], DT.bfloat16)
                nc.scalar.copy(tb[:], stgs[t // 8][:, t % 8, :])
                pp = s1p.tile([128, 128], DT.bfloat16)
                nc.tensor.transpose(pp[:], tb[:], ident[:])
                nc.vector.tensor_copy(
                    xa[:, n, 1 + h0 : 1 + h0 + 4, 1:33],
                    pp[:].rearrange("p (a b) -> p a b", a=4),
                )
        # ---- main loop
        # offsets handled by ScalarE (true |x-w|, +1 one-hot) vs VectorE
        # (min(x-w,0), -2 one-hot + sum(x) correction streams)
        act_set = set(range(9 - ACT_OFFS, 9))
        dve_offs = [oi for oi in range(9) if oi not in act_set]
        act_offs = [oi for oi in range(9) if oi in act_set]
        with tc.tile_pool(name="mp", bufs=1, space="PSUM") as mp, tc.tile_pool(
            name="udve", bufs=9
        ) as updve, tc.tile_pool(name="uact", bufs=9) as upact:
            P = mp.tile([128, M], DT.float32)

            # sum_c x streams (ones stationary), only for DVE-handled offsets;
            # the mixed offset covers only filter rows 0..95
            for k, oi in enumerate(dve_offs):
                i, j = OFFS[oi]
                for s in range(NCH):
                    n, h0 = divmod(s, 2)
                    nc.tensor.matmul(
                        P[:, s * CH : (s + 1) * CH],
                        ones96[:] if oi == mix_off else ones_s[:],
                        xa[:, n, i + h0 * 16 : i + h0 * 16 + 16, j : j + 32],
                        start=(k == 0),
                        stop=False,
                    )

            # filter group g covers filters {g, g+32, g+64, g+96}; strip js
            # computes filter g + 32*js as a 128x32 column-tiled matmul whose
            # PSUM partition range starts at 32*js, so PSUM row == filter.
            # Interleave ScalarE-class offsets among VectorE-class ones so the
            # PE queue roughly matches production-completion order.
            plan = list(dve_offs)
            for k, oi in enumerate(act_offs):
                plan.insert(3 + k * 4, oi)
            for g in range(FG):
                for oi in plan:
                    klass = "act" if oi in act_set else "dve"
                    if True:
                        i, j = OFFS[oi]
                        xin = xa[:, :, i : i + 32, :].rearrange(
                            "p n r c -> p n (r c)"
                        )
                        us = []
                        kls = []
                        for js in range(NSTRIP):
                            f = g + FG * js
                            wap = wt[:, oi, f : f + 1]
                            k2 = klass
                            if oi == mix_off and js == NSTRIP - 1:
                                k2 = "act"  # rebalance: ScalarE takes strip 3
                            pool = upact if k2 == "act" else updve
                            u = pool.tile([128, NL, 32 * PW], DT.bfloat16)
                            if k2 == "act":
                                nc.scalar.activation(
                                    u[:], xin, AF.Abs, bias=wap, scale=-1.0
                                )
                            else:
                                nc.vector.tensor_scalar(
                                    u[:], xin, wap, 0.0, AL.subtract, AL.min
                                )
                            us.append(u)
                            kls.append(k2)
                        last = g == FG - 1 and oi == plan[-1]
                        for s in range(NCH):
                            n, h0 = divmod(s, 2)
                            for js in range(NSTRIP):
                                zwin = zpos if kls[js] == "act" else zneg
                                stat = zwin[:, 127 - g : 159 - g]
                                u4 = us[js][:].rearrange(
                                    "p n (r c) -> p n r c", r=32
                                )
                                nc.tensor.matmul(
                                    P[32 * js : 32 * js + 32, s * CH : (s + 1) * CH],
                                    stat,
                                    u4[:, n, h0 * 16 : h0 * 16 + 16, j : j + 32],
                                    start=False,
                                    stop=last,
                                    tile_position=(0, 32 * js),
                                )

            # ---- drain: out = -P + (bias + sum w)
            sout = const.tile([128, M], DT.float32)
            for s in range(NCH):
                nc.scalar.activation(
                    sout[:, s * CH : (s + 1) * CH],
                    P[:, s * CH : (s + 1) * CH],
                    AF.Identity,
                    bias=bsum[:],
                    scale=-1.0,
                )

        # ---- transpose [f, m] -> [m, f] and store
        o_flat = o_d.rearrange("n h w f -> (n h w) f")
        with tc.tile_pool(name="tr", bufs=6) as trp, tc.tile_pool(
            name="trp", bufs=6, space="PSUM"
        ) as trpp:
            dma_engines = [nc.sync, nc.gpsimd, nc.scalar]
            for t in range(M // 128):
                pt = trpp.tile([128, 128], DT.float32)
                nc.tensor.transpose(pt[:], sout[:, t * 128 : (t + 1) * 128], ident32[:])
                ot = trp.tile([128, 128], DT.float32)
                nc.vector.tensor_copy(ot[:], pt[:])
                dma_engines[t % 3].dma_start(o_flat[t * 128 : (t + 1) * 128, :], ot[:])


_nc_cache = None


def _build():
    global _nc_cache
    if _nc_cache is None:
        nc = bacc.Bacc("TRN2", target_bir_lowering=False, debug=False, num_devices=N_CORES)
        x_d = nc.dram_tensor("inputs", [NL, H, W, C], DT.float32, kind="ExternalInput").ap()
        w_d = nc.dram_tensor("kernel", [3, 3, C, F], DT.float32, kind="ExternalInput").ap()
        b_d = nc.dram_tensor("bias", [1, F], DT.float32, kind="ExternalInput").ap()
        o_d = nc.dram_tensor("out", [NL, H, W, F], DT.float32, kind="ExternalOutput").ap()
        with tile.TileContext(nc) as tc:
            _body(tc, o_d, x_d, w_d, b_d)
        nc.compile()
        _nc_cache = nc
    return _nc_cache


def run(inputs, kernel, bias, **spmd_kwargs):
    nc = _build()
    shards = np.split(np.ascontiguousarray(inputs, dtype=np.float32), N_CORES, axis=0)
    kf = np.ascontiguousarray(kernel, dtype=np.float32)
    bf = np.ascontiguousarray(bias, dtype=np.float32).reshape(1, F)
    in_maps = [{"inputs": s, "kernel": kf, "bias": bf} for s in shards]
    res = run_bass_kernel_spmd(nc, in_maps, core_ids=list(range(N_CORES)), **spmd_kwargs)
    out = np.concatenate([r["out"] for r in res.results], axis=0)
    return out, res


def kernel(inputs, kernel, bias):
    out, _ = run(inputs, kernel, bias)
    return out
